# revision 8
# baseline (speedup 1.0000x reference)
"""DETM nelbo kernel for 8 Trainium2 NeuronCores.

Sharding: vocabulary V=30000 split 8 ways (3750/core).
 - theta-MLP first layer contracts over V  -> per-core partial + AllReduce.
 - beta path: logit[t,k,v] = alphas.rho  -> each core owns a V-slice; the
   softmax denominator Z[t,k] = sum_v exp(logit) is AllReduced.
 - nll: mix[b,v] = sum_k theta[b,k]/Z[t_b,k] * exp(logit[t_b,k,v]) computed
   per V-slice via a (time-masked theta) x explogit matmul; per-doc partial
   nll summed on host across cores.
Small sequential chains (alpha reparam + KLs, LSTM, eta chain) are replicated
on the host in fp32 numpy (they are O(T*K*E) / O(T*H^2), ~1e-3 of the FLOPs).
"""
import sys

if "/opt/trn_rl_repo" not in sys.path:
    sys.path.insert(0, "/opt/trn_rl_repo")

import numpy as np

import concourse.bass as bass
import concourse.mybir as mybir
import concourse.tile as tile
from concourse import bacc, bass_utils
from concourse.masks import make_identity

F32 = mybir.dt.float32
F32R = mybir.dt.float32r
BF16 = mybir.dt.bfloat16
AF = mybir.ActivationFunctionType
OP = mybir.AluOpType

V, K, E, T, B = 30000, 50, 300, 60, 128
TH, H, L = 800, 200, 3
NCORES = 8
VS = V // NCORES          # 3750
TK = T * K                # 3000
DELTA = 0.005

_CACHE = {}


def _build_program():
    nc = bacc.Bacc("TRN2", target_bir_lowering=False, debug=False,
                   num_devices=NCORES)

    def din(name, shape, dt=F32):
        return nc.dram_tensor(name, shape, dt, kind="ExternalInput").ap()

    nbT = din("nbT", [VS, B], F32R)            # normalized_bows shard, transposed
    w1vT = din("w1vT", [VS, TH], F32R)         # W1[:, vshard].T
    rhoT = din("rhoT", [E, VS], F32R)          # rho[vshard].T
    alphasT = din("alphasT", [E, TK], F32R)    # alphas as [E, T*K]
    bowsS = din("bowsS", [B, VS])
    etaC = din("etaC", [B, TH])                # eta_td @ W1[:, V:].T + b1
    etaTD = din("etaTD", [B, K])
    epsTH = din("epsTH", [B, K])
    onehotT = din("onehotT", [T, B])
    maskT = din("maskT", [100, 30 * B])        # time mask, 2-timestep chunks
    w2T = din("w2T", [TH, TH])
    wmulsT = din("wmulsT", [TH, 2 * K])
    b2R = din("b2R", [B, 7])
    bmulsB = din("bmulsB", [B, 2 * K])

    nllOut = nc.dram_tensor("nllOut", [B, 1], F32, kind="ExternalOutput").ap()
    klthOut = nc.dram_tensor("klthOut", [B, 1], F32, kind="ExternalOutput").ap()
    zmatOut = nc.dram_tensor("zmatOut", [T, K], F32, kind="ExternalOutput").ap()
    gOut = nc.dram_tensor("gOut", [100, 30 * B], F32, kind="ExternalOutput").ap()
    mixOut = nc.dram_tensor("mixOut", [B, VS], F32, kind="ExternalOutput").ap()
    thzOut = nc.dram_tensor("thzOut", [B, K], F32, kind="ExternalOutput").ap()

    KJ = [128, 128, 128, 128, 128, 128, 32]    # 800 split
    EK = [128, 128, 44]                        # 300 split
    RKL = np.float32(0.5 / (1.0 + 1e-6))       # 0.5/(exp(0)+1e-6) for kl_theta

    with tile.TileContext(nc) as tc:
        with tc.tile_pool(name="outer", bufs=1) as outer, \
             tc.tile_pool(name="dramp", bufs=1, space="DRAM") as dram:
            ident = outer.tile([128, 128], F32)
            make_identity(nc, ident[:])
            onehot_sb = outer.tile([T, B], F32)
            nc.sync.dma_start(onehot_sb[:], onehotT[:])
            gT_sb = outer.tile([100, 30, B], BF16)
            zmat_sb = outer.tile([T, K], F32)
            nllp = outer.tile([B, 8], F32)
            nc.vector.memset(nllp[:], 0.0)

            ar1_in = dram.tile([B, TH], F32)
            ar1_out = dram.tile([B, TH], F32, addr_space="Shared")
            z_in = dram.tile([3072], F32)
            z_out = dram.tile([3072], F32, addr_space="Shared")
            explog_d = dram.tile([3072, VS], BF16)

            # ---------------- Phase 1: h1_pre partial + AllReduce --------
            with tc.tile_pool(name="p1nb", bufs=3) as p1nb, \
                 tc.tile_pool(name="p1w", bufs=3) as p1w, \
                 tc.tile_pool(name="p1s", bufs=1) as p1s, \
                 tc.tile_pool(name="p1ps", bufs=1, space="PSUM") as p1ps:
                psA = p1ps.tile([128, 2, 512], F32)
                for c in range(30):
                    r0 = c * 125
                    nb_t = p1nb.tile([125, B], F32R, name="nb_t")
                    nc.sync.dma_start(nb_t[:], nbT[r0:r0 + 125, :])
                    w1_t = p1w.tile([125, TH], F32R, name="w1_t")
                    nc.sync.dma_start(w1_t[:], w1vT[r0:r0 + 125, :])
                    st = c == 0
                    sp = c == 29
                    nc.tensor.matmul(psA[:, 0, :400], nb_t[:], w1_t[:, :400],
                                     start=st, stop=sp)
                    nc.tensor.matmul(psA[:, 1, :400], nb_t[:], w1_t[:, 400:],
                                     start=st, stop=sp)
                h1preS = p1s.tile([B, TH], F32)
                nc.vector.tensor_copy(h1preS[:], psA[:, :, :400])
                nc.sync.dma_start(ar1_in[:], h1preS[:])
            nc.gpsimd.collective_compute(
                "AllReduce", OP.add,
                replica_groups=[list(range(NCORES))],
                ins=[ar1_in[:].opt()], outs=[ar1_out[:].opt()])

            # ---------------- Phase 2: MLP + einsum + Z + G --------------
            with tc.tile_pool(name="p2", bufs=1) as p2, \
                 tc.tile_pool(name="p2ps", bufs=2, space="PSUM") as p2ps, \
                 tc.tile_pool(name="p2eo", bufs=3) as p2eo, \
                 tc.tile_pool(name="p2mps", bufs=2, space="PSUM") as p2mps:
                # --- MLP (waits on AR1; emitted first for priority) ---
                h1pre_sb = p2.tile([B, TH], F32)
                nc.sync.dma_start(h1pre_sb[:], ar1_out[:])
                etaC_sb = p2.tile([B, TH], F32)
                nc.sync.dma_start(etaC_sb[:], etaC[:])
                hsum = p2.tile([B, TH], F32)
                nc.vector.tensor_add(hsum[:], h1pre_sb[:], etaC_sb[:])
                h1_sb = p2.tile([B, TH], F32)
                nc.scalar.activation(h1_sb[:], hsum[:], AF.Relu)

                b2R_sb = p2.tile([B, 7], F32)
                nc.sync.dma_start(b2R_sb[:], b2R[:])
                h1T = p2.tile([128, 7, B], F32)
                for j in range(7):
                    w = KJ[j]
                    pt = p2mps.tile([128, 512], F32, name="pt", tag="mps")
                    nc.tensor.transpose(pt[:w, :128], h1_sb[:, j * 128:j * 128 + w],
                                        ident[:])
                    nc.vector.tensor_copy(h1T[:w, j, :], pt[:w, :128])

                w2T_sb = p2.tile([128, 7, TH], F32)
                for j in range(7):
                    nc.sync.dma_start(w2T_sb[:KJ[j], j, :],
                                      w2T[j * 128:j * 128 + KJ[j], :])
                h2T = p2.tile([128, 7, B], F32)
                for jo in range(7):
                    wjo = KJ[jo]
                    psH = p2mps.tile([128, 512], F32, name="psH", tag="mps")
                    for ji in range(7):
                        nc.tensor.matmul(
                            psH[:wjo, :128],
                            w2T_sb[:KJ[ji], ji, jo * 128:jo * 128 + wjo],
                            h1T[:KJ[ji], ji, :],
                            start=(ji == 0), stop=(ji == 6))
                    nc.scalar.activation(h2T[:wjo, jo, :], psH[:wjo, :128],
                                         AF.Relu, bias=b2R_sb[:wjo, jo:jo + 1])

                wmuls_sb = p2.tile([128, 7, 2 * K], F32)
                for j in range(7):
                    nc.sync.dma_start(wmuls_sb[:KJ[j], j, :],
                                      wmulsT[j * 128:j * 128 + KJ[j], :])
                psM = p2mps.tile([128, 512], F32, name="psM", tag="mps")
                for ji in range(7):
                    nc.tensor.matmul(psM[:, :2 * K], h2T[:KJ[ji], ji, :],
                                     wmuls_sb[:KJ[ji], ji, :],
                                     start=(ji == 0), stop=(ji == 6))
                bmuls_sb = p2.tile([B, 2 * K], F32)
                nc.sync.dma_start(bmuls_sb[:], bmulsB[:])
                muls = p2.tile([B, 2 * K], F32)
                nc.vector.tensor_add(muls[:], psM[:, :2 * K], bmuls_sb[:])
                mu = muls[:, :K]
                ls = muls[:, K:]
                sd = p2.tile([B, K], F32)
                nc.scalar.activation(sd[:], ls, AF.Exp, scale=0.5)
                epsTH_sb = p2.tile([B, K], F32)
                nc.sync.dma_start(epsTH_sb[:], epsTH[:])
                ez0 = p2.tile([B, K], F32)
                nc.vector.tensor_mul(ez0[:], epsTH_sb[:], sd[:])
                zt = p2.tile([B, K], F32)
                nc.vector.tensor_add(zt[:], mu, ez0[:])
                zm = p2.tile([B, 1], F32)
                nc.vector.reduce_max(zm[:], zt[:], axis=mybir.AxisListType.X,
                                     negate=True)
                et = p2.tile([B, K], F32)
                se = p2.tile([B, 1], F32)
                nc.scalar.activation(et[:], zt[:], AF.Exp, bias=zm[:],
                                     accum_out=se[:])
                rse = p2.tile([B, 1], F32)
                nc.vector.reciprocal(rse[:], se[:])
                theta = p2.tile([B, K], F32)
                nc.vector.tensor_scalar_mul(theta[:], et[:], rse[:])

                # kl_theta (replicated)
                etaTD_sb = p2.tile([B, K], F32)
                nc.sync.dma_start(etaTD_sb[:], etaTD[:])
                sd2 = p2.tile([B, K], F32)
                nc.vector.tensor_mul(sd2[:], sd[:], sd[:])
                dd = p2.tile([B, K], F32)
                nc.vector.tensor_sub(dd[:], mu, etaTD_sb[:])
                dd2 = p2.tile([B, K], F32)
                nc.vector.tensor_mul(dd2[:], dd[:], dd[:])
                uu = p2.tile([B, K], F32)
                sA = p2.tile([B, 1], F32)
                nc.vector.scalar_tensor_tensor(uu[:], dd2[:], 1.0, sd2[:],
                                               op0=OP.bypass, op1=OP.add,
                                               accum_out=sA[:])
                sB_ = p2.tile([B, 1], F32)
                nc.vector.reduce_sum(sB_[:], ls, axis=mybir.AxisListType.X)
                q1 = p2.tile([B, 1], F32)
                nc.vector.tensor_scalar(q1[:], sA[:], float(RKL), -float(K) * 0.5,
                                        op0=OP.mult, op1=OP.add)
                q2 = p2.tile([B, 1], F32)
                nc.vector.tensor_scalar_mul(q2[:], sB_[:], 0.5)
                klth = p2.tile([B, 1], F32)
                nc.vector.tensor_sub(klth[:], q1[:], q2[:])
                nc.sync.dma_start(klthOut[:], klth[:])

                # --- einsum: explogit + Z partials ---
                rho_sb = p2.tile([128, 3, VS], F32R)
                for kc in range(3):
                    nc.sync.dma_start(rho_sb[:EK[kc], kc, :],
                                      rhoT[kc * 128:kc * 128 + EK[kc], :])
                alph_sb = p2.tile([128, 3, TK], F32R)
                for kc in range(3):
                    nc.sync.dma_start(alph_sb[:EK[kc], kc, :],
                                      alphasT[kc * 128:kc * 128 + EK[kc], :])
                zparts = p2.tile([128, 72], F32)
                nc.vector.memset(zparts[:], 0.0)
                NCH = [(0, 1536), (1536, 1536), (3072, 678)]
                for m in range(24):
                    mrows = 128 if m < 23 else 56
                    m0 = m * 128
                    for nch, (n0, nw) in enumerate(NCH):
                        ps = p2ps.tile([128, 1536], F32, name="psC")
                        nsub = (nw + 511) // 512
                        for s in range(nsub):
                            w = min(512, nw - s * 512)
                            for kc in range(3):
                                nc.tensor.matmul(
                                    ps[:mrows, s * 512:s * 512 + w],
                                    alph_sb[:EK[kc], kc, m0:m0 + mrows],
                                    rho_sb[:EK[kc], kc, n0 + s * 512:n0 + s * 512 + w],
                                    start=(kc == 0), stop=(kc == 2))
                        eo = p2eo.tile([128, 1536], BF16, name="eo")
                        nc.scalar.activation(eo[:mrows, :nw], ps[:mrows, :nw],
                                             AF.Exp,
                                             accum_out=zparts[:mrows,
                                                              m * 3 + nch:m * 3 + nch + 1])
                        if mrows > 100:
                            nc.sync.dma_start(explog_d[m0:m0 + 100, n0:n0 + nw],
                                              eo[:100, :nw])
                            nc.sync.dma_start(
                                explog_d[m0 + 100:m0 + mrows, n0:n0 + nw],
                                eo[100:mrows, :nw])
                        else:
                            nc.sync.dma_start(explog_d[m0:m0 + mrows, n0:n0 + nw],
                                              eo[:mrows, :nw])
                zred = p2.tile([128, 24], F32)
                nc.vector.reduce_sum(
                    zred[:], zparts[:].rearrange("p (m c) -> p m c", c=3),
                    axis=mybir.AxisListType.X)
                nc.sync.dma_start(z_in[:].rearrange("(a b) -> b a", b=128),
                                  zred[:])
                nc.gpsimd.collective_compute(
                    "AllReduce", OP.add,
                    replica_groups=[list(range(NCORES))],
                    ins=[z_in[:].opt()], outs=[z_out[:].opt()])
                nc.sync.dma_start(zmat_sb[:],
                                  z_out[0:TK].rearrange("(t k) -> t k", k=K))
                nc.sync.dma_start(zmatOut[:], zmat_sb[:])

                # --- Z_td gather, thetaZ, G ---
                psZ = p2mps.tile([128, 512], F32, name="psZ", tag="mps")
                nc.tensor.matmul(psZ[:, :K], onehot_sb[:], zmat_sb[:],
                                 start=True, stop=True)
                rz = p2.tile([B, K], F32)
                nc.vector.reciprocal(rz[:], psZ[:, :K])
                thz = p2.tile([B, K], F32)
                nc.vector.tensor_mul(thz[:], theta[:], rz[:])
                nc.sync.dma_start(thzOut[:], thz[:])
                thzp = p2.tile([128, 128], F32)
                nc.vector.memset(thzp[:], 0.0)
                nc.vector.tensor_copy(thzp[:, :K], thz[:])
                psG = p2mps.tile([128, 512], F32, name="psG", tag="mps")
                nc.tensor.transpose(psG[:, :128], thzp[:], ident[:])
                thzT = p2.tile([K, B], F32)
                nc.vector.tensor_copy(thzT[:], psG[0:K, :128])
                rep = p2.tile([100, B], F32)
                nc.sync.dma_start(rep[0:K, :], thzT[:])
                nc.sync.dma_start(rep[K:2 * K, :], thzT[:])
                maskT_sb = p2.tile([100, 30 * B], F32)
                nc.sync.dma_start(maskT_sb[:], maskT[:])
                nc.vector.tensor_mul(
                    gT_sb[:],
                    rep[:].unsqueeze(1).broadcast_to((100, 30, B)),
                    maskT_sb[:].rearrange("p (c b) -> p c b", b=B))
                gf32 = p2.tile([100, 30 * B], F32)
                nc.vector.tensor_copy(gf32[:], gT_sb[:].rearrange("p c b -> p (c b)"))
                nc.sync.dma_start(gOut[:], gf32[:])

            # ---------------- Phase 3: mix matmul + nll ------------------
            with tc.tile_pool(name="p3", bufs=1) as p3, \
                 tc.tile_pool(name="p3ex", bufs=3) as p3ex, \
                 tc.tile_pool(name="p3lm", bufs=2) as p3lm, \
                 tc.tile_pool(name="p3ps", bufs=1, space="PSUM") as p3ps:
                bows_sb = p3.tile([B, VS], F32)
                nc.sync.dma_start(bows_sb[:], bowsS[:])
                eps6 = p3.tile([B, 1], F32)
                nc.vector.memset(eps6[:], 1e-6)
                NW = [512] * 7 + [166]
                psd = [p3ps.tile([128, 512], F32, name=f"psd{i}")
                       for i in range(8)]
                for c in range(30):
                    ex_t = p3ex.tile([100, VS], BF16, name="ex_t")
                    nc.sync.dma_start(ex_t[:], explog_d[c * 100:c * 100 + 100, :])
                    for n in range(8):
                        w = NW[n]
                        nc.tensor.matmul(psd[n][:, :w], gT_sb[:, c, :],
                                         ex_t[:, n * 512:n * 512 + w],
                                         start=(c == 0), stop=(c == 29))
                for n in range(8):
                    w = NW[n]
                    mixs = p3lm.tile([B, 512], F32, name="mixs")
                    nc.vector.tensor_copy(mixs[:, :w], psd[n][:, :w])
                    nc.sync.dma_start(mixOut[:, n * 512:n * 512 + w],
                                      mixs[:, :w])
                    lm = p3lm.tile([B, 512], F32, name="lm")
                    nc.scalar.activation(lm[:, :w], psd[n][:, :w], AF.Ln,
                                         bias=eps6[:])
                    junk = p3lm.tile([B, 512], F32, name="junk")
                    nc.vector.scalar_tensor_tensor(
                        junk[:, :w], lm[:, :w], 1.0,
                        bows_sb[:, n * 512:n * 512 + w],
                        op0=OP.bypass, op1=OP.mult,
                        accum_out=nllp[:, n:n + 1])
                nsum = p3.tile([B, 1], F32)
                nc.vector.reduce_sum(nsum[:], nllp[:], axis=mybir.AxisListType.X,
                                     negate=True)
                nc.sync.dma_start(nllOut[:], nsum[:])

    nc.compile()
    return nc


# ---------------------------------------------------------------------------
# host-side small sequential chains (fp32 numpy)
# ---------------------------------------------------------------------------

def _sigmoid(x):
    with np.errstate(over="ignore"):
        return (1.0 / (1.0 + np.exp(-x))).astype(np.float32)


def _kl_np(qm, qls, pm, pls):
    return 0.5 * np.sum(
        (np.exp(qls) + (qm - pm) ** 2) / (np.exp(pls) + 1e-6)
        - 1.0 + pls - qls, axis=-1, dtype=np.float32)


def _host_chains(inp):
    f = np.float32
    mu_a = np.asarray(inp["mu_q_alpha"], f).transpose(1, 0, 2)
    ls_a = np.asarray(inp["logsigma_q_alpha"], f).transpose(1, 0, 2)
    eps_a = np.asarray(inp["eps_alpha"], f)
    logdelta = f(np.log(f(DELTA)))
    alphas = (mu_a + eps_a * np.exp(0.5 * ls_a)).astype(f)
    kl_alpha = f(_kl_np(mu_a[0], ls_a[0], f(0.0), f(0.0)).sum()
                 + _kl_np(mu_a[1:], ls_a[1:], alphas[:-1], logdelta).sum())

    rnn_inp = np.asarray(inp["rnn_inp"], f)
    Wmap = np.asarray(inp["Wmap"], f)
    bmap = np.asarray(inp["bmap"], f)
    out = (rnn_inp @ Wmap.T + bmap).astype(f)
    Wih = np.asarray(inp["lstm_Wih"], f)
    Whh = np.asarray(inp["lstm_Whh"], f)
    bih = np.asarray(inp["lstm_bih"], f)
    bhh = np.asarray(inp["lstm_bhh"], f)
    for l in range(L):
        h = np.zeros(H, f)
        c = np.zeros(H, f)
        pre = (out @ Wih[l].T + (bih[l] + bhh[l])).astype(f)
        ys = np.empty((T, H), f)
        for t in range(T):
            g = pre[t] + Whh[l] @ h
            i_, f_, g_, o_ = np.split(g, 4)
            c = _sigmoid(f_) * c + _sigmoid(i_) * np.tanh(g_)
            h = (_sigmoid(o_) * np.tanh(c)).astype(f)
            ys[t] = h
        out = ys
    Wmu_e = np.asarray(inp["Wmu_e"], f)
    bmu_e = np.asarray(inp["bmu_e"], f)
    Wls_e = np.asarray(inp["Wls_e"], f)
    bls_e = np.asarray(inp["bls_e"], f)
    eps_eta = np.asarray(inp["eps_eta"], f)
    inp0 = np.concatenate([out[0], np.zeros(K, f)])
    mu0 = Wmu_e @ inp0 + bmu_e
    ls0 = Wls_e @ inp0 + bls_e
    eta = mu0 + eps_eta[0] * np.exp(0.5 * ls0)
    kl_eta = _kl_np(mu0, ls0, f(0.0), f(0.0))
    etas = np.empty((T, K), f)
    etas[0] = eta
    for t in range(1, T):
        it = np.concatenate([out[t], eta])
        mu_t = Wmu_e @ it + bmu_e
        ls_t = Wls_e @ it + bls_e
        eta = (mu_t + eps_eta[t] * np.exp(0.5 * ls_t)).astype(f)
        kl_eta = kl_eta + _kl_np(mu_t, ls_t, etas[t - 1], logdelta)
        etas[t] = eta
    return alphas, f(kl_alpha), etas, f(kl_eta)


def kernel(**inputs):
    f = np.float32
    if "nc" not in _CACHE:
        _CACHE["nc"] = _build_program()
    nc = _CACHE["nc"]

    bows = np.asarray(inputs["bows"], f)
    nb = np.asarray(inputs["normalized_bows"], f)
    times = np.asarray(inputs["times"]).astype(np.int64)
    num_docs = float(np.asarray(inputs["num_docs"]))
    W1 = np.asarray(inputs["W1"], f)
    b1 = np.asarray(inputs["b1"], f)
    W2 = np.asarray(inputs["W2"], f)
    b2 = np.asarray(inputs["b2"], f)
    Wmu_t = np.asarray(inputs["Wmu_t"], f)
    bmu_t = np.asarray(inputs["bmu_t"], f)
    Wls_t = np.asarray(inputs["Wls_t"], f)
    bls_t = np.asarray(inputs["bls_t"], f)
    rho = np.asarray(inputs["rho"], f)
    eps_theta = np.asarray(inputs["eps_theta"], f)

    alphas, kl_alpha, etas, kl_eta = _host_chains(inputs)
    eta_td = etas[times]                                   # [B, K]
    etaC = (eta_td @ W1[:, V:].T + b1).astype(f)           # [B, TH]

    onehotT = (times[None, :] == np.arange(T)[:, None]).astype(f)
    pp = np.arange(100) // K
    tgrid = 2 * np.arange(30)[None, :] + pp[:, None]       # [100, 30]
    maskT = (times[None, None, :] == tgrid[:, :, None]).astype(f)
    maskT = np.ascontiguousarray(maskT.reshape(100, 30 * B))

    alphasT = np.ascontiguousarray(alphas.reshape(TK, E).T)
    w2T = np.ascontiguousarray(W2.T)
    wmulsT = np.ascontiguousarray(np.concatenate([Wmu_t, Wls_t], axis=0).T)
    pad = np.zeros(896, f)
    pad[:TH] = b2
    b2R = np.ascontiguousarray(pad.reshape(7, 128).T)
    bmulsB = np.ascontiguousarray(
        np.broadcast_to(np.concatenate([bmu_t, bls_t]).astype(f), (B, 2 * K)))

    in_maps = []
    for c in range(NCORES):
        sl = slice(c * VS, (c + 1) * VS)
        in_maps.append({
            "nbT": np.ascontiguousarray(nb[:, sl].T),
            "w1vT": np.ascontiguousarray(W1[:, sl].T),
            "rhoT": np.ascontiguousarray(rho[sl, :].T),
            "alphasT": alphasT,
            "bowsS": np.ascontiguousarray(bows[:, sl]),
            "etaC": etaC,
            "etaTD": np.ascontiguousarray(eta_td.astype(f)),
            "epsTH": eps_theta,
            "onehotT": onehotT,
            "maskT": maskT,
            "w2T": w2T,
            "wmulsT": wmulsT,
            "b2R": b2R,
            "bmulsB": bmulsB,
        })

    global _LAST_IN_MAPS
    _LAST_IN_MAPS = in_maps
    res = bass_utils.run_bass_kernel_spmd(nc, in_maps,
                                          core_ids=list(range(NCORES)))
    _CACHE["res"] = res
    coeff = f(num_docs / B)
    nll_tot = f(sum(r["nllOut"].sum(dtype=np.float64) for r in res.results))
    nll_tot = f(nll_tot * coeff)
    klth_tot = f(res.results[0]["klthOut"].sum(dtype=np.float64) * coeff)
    nelbo = f(nll_tot + kl_alpha + kl_eta + klth_tot)
    return np.array([nelbo, nll_tot, kl_alpha, kl_eta, klth_tot], dtype=f)



# revision 21
# speedup vs baseline: 1.6511x; 1.6511x over previous
"""DETM nelbo kernel for 8 Trainium2 NeuronCores.

Sharding: vocabulary V=30000 split 8 ways (3750/core).
 - theta-MLP first layer contracts over V  -> per-core partial + AllReduce
   (computed transposed: h1preT [TH, B], so no on-chip transposes needed).
 - beta path: logit[t,k,v] = alphas.rho per V-slice; explog = exp(logit-ln s)
   is kept fully SBUF-resident in fp8e5 (e5m2). The softmax denominator
   Z[t,k] = s * sum_v exp(logit - ln s) is accumulated in f32 alongside the
   Exp activation and AllReduced.
 - nll: mix[b,v] = 2^q * sum_k theta[b,k]/Z[t_b,k] * explog[t_b,k,v] via
   24 x [128-row-chunk] fp8 matmuls: weights G2[m][p,b] =
   thz[b, p%50] * s * 2^q * [times[b] == row//50], built on-device from a
   tiny mask matmul; psum accumulates over all chunks.  loglik =
   Ln(psum + 1e-6*2^q) - q ln2; the -q ln2 * sum(bows) correction is applied
   on the host.
Small sequential chains (alpha reparam + KLs, LSTM, eta chain) are replicated
on the host in fp32 numpy (they are O(T*K*E) / O(T*H^2), ~1e-3 of the FLOPs).
"""
import sys

if "/opt/trn_rl_repo" not in sys.path:
    sys.path.insert(0, "/opt/trn_rl_repo")

import numpy as np
import ml_dtypes

import concourse.bass as bass
import concourse.mybir as mybir
import concourse.tile as tile
from concourse import bacc, bass_utils
from concourse.masks import make_identity

F32 = mybir.dt.float32
BF16 = mybir.dt.bfloat16
FP8 = mybir.dt.float8e5
AF = mybir.ActivationFunctionType
OP = mybir.AluOpType

V, K, E, T, B = 30000, 50, 300, 60, 128
TH, H, L = 800, 200, 3
NCORES = 8
VS = V // NCORES          # 3750
TK = T * K                # 3000
DELTA = 0.005

NCH = [(0, 1536), (1536, 1536), (3072, 678)]   # einsum column chunks
EK = [128, 128, 44]                            # E=300 contraction chunks
KJ = [128, 128, 128, 128, 128, 128, 32]        # TH=800 chunks
MCH = [128] * 23 + [56]                        # TK=3000 row chunks
P1CH = [128] * 29 + [38]                       # VS=3750 row chunks
NW = [512] * 7 + [166]                         # VS mix column chunks

_CACHE = {}


def _build_program():
    nc = bacc.Bacc("TRN2", target_bir_lowering=False, debug=False,
                   num_devices=NCORES)

    def din(name, shape, dt=F32):
        return nc.dram_tensor(name, shape, dt, kind="ExternalInput").ap()

    nbT = din("nbT", [VS, B], BF16)            # normalized_bows shard^T
    w1vT = din("w1vT", [VS, TH], BF16)         # W1[:, vshard].T
    rhoT = din("rhoT", [E, VS], BF16)          # rho[vshard].T
    alphasT = din("alphasT", [E, TK], BF16)    # alphas as [E, T*K]
    bowsS = din("bowsS", [B, VS], BF16)
    etaC = din("etaC", [B, TH])                # eta_td @ W1[:,V:].T + b1
    etaTD = din("etaTD", [B, K])
    epsTH = din("epsTH", [B, K])
    onehotT = din("onehotT", [T, B])           # f32, for Z gather matmul
    F2m = din("F2m", [128, 24 * B], BF16)      # time mask by (row, chunk, doc)
    RTm = din("RTm", [50, 24 * 128], BF16)     # k-cyclic selection matrix
    w2T = din("w2T", [128, 7, TH], BF16)       # W2.T chunked [i-part, ichunk, o]
    wmulsT = din("wmulsT", [128, 7, 2 * K], BF16)
    b2R = din("b2R", [128, 7])
    bmulsB = din("bmulsB", [B, 2 * K])
    scal4 = din("scal4", [128, 4])             # cols: -ln s, 1e-6*2^q, s, s*2^q

    nllOut = nc.dram_tensor("nllOut", [B, 1], F32, kind="ExternalOutput").ap()
    klthOut = nc.dram_tensor("klthOut", [B, 1], F32, kind="ExternalOutput").ap()
    zmatOut = nc.dram_tensor("zmatOut", [T, K], F32, kind="ExternalOutput").ap()
    thzOut = nc.dram_tensor("thzOut", [B, K], F32, kind="ExternalOutput").ap()

    RKL = np.float32(0.5 / (1.0 + 1e-6))

    with tile.TileContext(nc) as tc:
        with tc.tile_pool(name="outer", bufs=1) as outer, \
             tc.tile_pool(name="dramp", bufs=1, space="DRAM") as dram:
            scal = outer.tile([128, 4], F32)
            nc.sync.dma_start(scal[:], scal4[:])
            rho_sb = outer.tile([128, 3, VS], BF16)
            for kc in range(3):
                nc.sync.dma_start(rho_sb[:EK[kc], kc, :],
                                  rhoT[kc * 128:kc * 128 + EK[kc], :])
            onehot_sb = outer.tile([T, B], F32)
            nc.sync.dma_start(onehot_sb[:], onehotT[:])
            F2_sb = outer.tile([128, 24, B], BF16)
            nc.sync.dma_start(F2_sb[:], F2m[:].rearrange("p (m b) -> p m b", b=B))
            RT_sb = outer.tile([50, 24, 128], BF16)
            nc.sync.dma_start(RT_sb[:], RTm[:].rearrange("k (m p) -> k m p", p=128))
            etaC_sb = outer.tile([B, TH], F32)
            nc.sync.dma_start(etaC_sb[:], etaC[:])
            ident = outer.tile([128, 128], BF16)
            make_identity(nc, ident[:])

            exs = outer.tile([128, 24, VS], FP8)      # resident explog/s
            G2 = outer.tile([128, 24, B], FP8)
            zparts = outer.tile([128, 72], F32)
            nc.vector.memset(zparts[:], 0.0)
            nllp = outer.tile([B, 8], F32)
            nc.vector.memset(nllp[:], 0.0)
            bows_sb = outer.tile([B, VS], BF16)
            nc.sync.dma_start(bows_sb[:], bowsS[:])

            ar1_in = dram.tile([B, TH], F32)
            ar1_out = dram.tile([B, TH], F32, addr_space="Shared")
            z_in = dram.tile([3072], F32)
            z_out = dram.tile([3072], F32, addr_space="Shared")

            # ---------------- Phase 1: h1preT partials + AllReduce -------
            with tc.tile_pool(name="p1in", bufs=3) as p1in, \
                 tc.tile_pool(name="p1s", bufs=1) as p1s, \
                 tc.tile_pool(name="p1ps", bufs=1, space="PSUM") as p1ps:
                psA = p1ps.tile([128, 2, 512], F32)
                r0 = 0
                for c, rs in enumerate(P1CH):
                    nb_t = p1in.tile([128, B], BF16, name="nb_t")
                    nc.sync.dma_start(nb_t[:rs], nbT[r0:r0 + rs, :])
                    w1_t = p1in.tile([128, TH], BF16, name="w1_t")
                    nc.sync.dma_start(w1_t[:rs], w1vT[r0:r0 + rs, :])
                    nc.tensor.matmul(psA[:, 0, :400], nb_t[:rs, :],
                                     w1_t[:rs, :400],
                                     start=(c == 0), stop=(c == 29))
                    nc.tensor.matmul(psA[:, 1, :400], nb_t[:rs, :],
                                     w1_t[:rs, 400:],
                                     start=(c == 0), stop=(c == 29))
                    r0 += rs
                h1preS = p1s.tile([B, TH], F32)
                nc.vector.tensor_copy(h1preS[:], psA[:, :, :400])
                nc.sync.dma_start(ar1_in[:], h1preS[:])
            nc.gpsimd.collective_compute(
                "AllReduce", OP.add,
                replica_groups=[list(range(NCORES))],
                ins=[ar1_in[:].opt()], outs=[ar1_out[:].opt()])

            # ---------------- Phase 2: einsum + exp (fp8, resident) ------
            with tc.tile_pool(name="peps", bufs=2, space="PSUM") as peps, \
                 tc.tile_pool(name="pa", bufs=3) as pa:
                m0 = 0
                for m, mrows in enumerate(MCH):
                    a_t = pa.tile([128, 3, 128], BF16, name="a_t")
                    for kc in range(3):
                        nc.sync.dma_start(
                            a_t[:EK[kc], kc, :mrows],
                            alphasT[kc * 128:kc * 128 + EK[kc], m0:m0 + mrows])
                    for nch, (n0, nw) in enumerate(NCH):
                        ps = peps.tile([128, 1536], F32, name="psE")
                        nsub = (nw + 511) // 512
                        for s in range(nsub):
                            w = min(512, nw - s * 512)
                            for kc in range(3):
                                nc.tensor.matmul(
                                    ps[:mrows, s * 512:s * 512 + w],
                                    a_t[:EK[kc], kc, :mrows],
                                    rho_sb[:EK[kc], kc,
                                           n0 + s * 512:n0 + s * 512 + w],
                                    start=(kc == 0), stop=(kc == 2))
                        nc.scalar.activation(
                            exs[:mrows, m, n0:n0 + nw], ps[:mrows, :nw],
                            AF.Exp, bias=scal[:mrows, 0:1],
                            accum_out=zparts[:mrows,
                                             m * 3 + nch:m * 3 + nch + 1])
                    m0 += mrows

                # ---------------- MLP (waits on AR1; fills PE gaps) ------
                with tc.tile_pool(name="pm", bufs=1) as pm, \
                     tc.tile_pool(name="pmps", bufs=2, space="PSUM") as pmps:
                    h1pre = pm.tile([B, TH], F32)
                    nc.sync.dma_start(h1pre[:], ar1_out[:])
                    hsum = pm.tile([B, TH], F32)
                    nc.vector.tensor_add(hsum[:], h1pre[:], etaC_sb[:])
                    h1b = pm.tile([B, TH], BF16)
                    nc.scalar.activation(h1b[:], hsum[:], AF.Relu)
                    h1T = pm.tile([128, 7, B], BF16)
                    for j in range(7):
                        ptp = pmps.tile([128, 512], BF16, name="ptp", tag="mps")
                        nc.tensor.transpose(ptp[:KJ[j], :128],
                                            h1b[:, j * 128:j * 128 + KJ[j]],
                                            ident[:])
                        nc.vector.tensor_copy(h1T[:KJ[j], j, :],
                                              ptp[:KJ[j], :128])

                    w2T_sb = pm.tile([128, 7, TH], BF16)
                    for j in range(7):
                        nc.sync.dma_start(w2T_sb[:KJ[j], j, :], w2T[:KJ[j], j, :])
                    b2R_sb = pm.tile([128, 7], F32)
                    nc.sync.dma_start(b2R_sb[:], b2R[:])
                    h2T = pm.tile([128, 7, B], BF16)
                    for jo in range(7):
                        wjo = KJ[jo]
                        psH = pmps.tile([128, 512], F32, name="psH", tag="mps")
                        for ji in range(7):
                            nc.tensor.matmul(
                                psH[:wjo, :B],
                                w2T_sb[:KJ[ji], ji, jo * 128:jo * 128 + wjo],
                                h1T[:KJ[ji], ji, :],
                                start=(ji == 0), stop=(ji == 6))
                        nc.scalar.activation(h2T[:wjo, jo, :], psH[:wjo, :B],
                                             AF.Relu, bias=b2R_sb[:wjo, jo:jo + 1])

                    wmuls_sb = pm.tile([128, 7, 2 * K], BF16)
                    nc.sync.dma_start(wmuls_sb[:], wmulsT[:])
                    psM = pmps.tile([128, 512], F32, name="psM", tag="mps")
                    for ji in range(7):
                        nc.tensor.matmul(psM[:B, :2 * K], h2T[:KJ[ji], ji, :],
                                         wmuls_sb[:KJ[ji], ji, :],
                                         start=(ji == 0), stop=(ji == 6))
                    bmuls_sb = pm.tile([B, 2 * K], F32)
                    nc.sync.dma_start(bmuls_sb[:], bmulsB[:])
                    muls = pm.tile([B, 2 * K], F32)
                    nc.vector.tensor_add(muls[:], psM[:B, :2 * K], bmuls_sb[:])
                    mu = muls[:, :K]
                    ls = muls[:, K:]
                    sd = pm.tile([B, K], F32)
                    nc.scalar.activation(sd[:], ls, AF.Exp, scale=0.5)
                    epsTH_sb = pm.tile([B, K], F32)
                    nc.sync.dma_start(epsTH_sb[:], epsTH[:])
                    ez0 = pm.tile([B, K], F32)
                    nc.vector.tensor_mul(ez0[:], epsTH_sb[:], sd[:])
                    zt = pm.tile([B, K], F32)
                    nc.vector.tensor_add(zt[:], mu, ez0[:])
                    zm = pm.tile([B, 1], F32)
                    nc.vector.reduce_max(zm[:], zt[:], axis=mybir.AxisListType.X,
                                         negate=True)
                    et = pm.tile([B, K], F32)
                    se = pm.tile([B, 1], F32)
                    nc.scalar.activation(et[:], zt[:], AF.Exp, bias=zm[:],
                                         accum_out=se[:])
                    rse = pm.tile([B, 1], F32)
                    nc.vector.reciprocal(rse[:], se[:])
                    theta = pm.tile([B, K], F32)
                    nc.vector.tensor_scalar_mul(theta[:], et[:], rse[:])

                    # kl_theta (replicated on every core; core 0's is used)
                    etaTD_sb = pm.tile([B, K], F32)
                    nc.sync.dma_start(etaTD_sb[:], etaTD[:])
                    sd2 = pm.tile([B, K], F32)
                    nc.vector.tensor_mul(sd2[:], sd[:], sd[:])
                    dd = pm.tile([B, K], F32)
                    nc.vector.tensor_sub(dd[:], mu, etaTD_sb[:])
                    dd2 = pm.tile([B, K], F32)
                    nc.vector.tensor_mul(dd2[:], dd[:], dd[:])
                    uu = pm.tile([B, K], F32)
                    sA = pm.tile([B, 1], F32)
                    nc.vector.scalar_tensor_tensor(uu[:], dd2[:], 1.0, sd2[:],
                                                   op0=OP.bypass, op1=OP.add,
                                                   accum_out=sA[:])
                    sB_ = pm.tile([B, 1], F32)
                    nc.vector.reduce_sum(sB_[:], ls, axis=mybir.AxisListType.X)
                    q1 = pm.tile([B, 1], F32)
                    nc.vector.tensor_scalar(q1[:], sA[:], float(RKL),
                                            -float(K) * 0.5,
                                            op0=OP.mult, op1=OP.add)
                    q2 = pm.tile([B, 1], F32)
                    nc.vector.tensor_scalar_mul(q2[:], sB_[:], 0.5)
                    klth = pm.tile([B, 1], F32)
                    nc.vector.tensor_sub(klth[:], q1[:], q2[:])
                    nc.sync.dma_start(klthOut[:], klth[:])

                    # ------------- Z AllReduce + thz + G2 ----------------
                    zred = pm.tile([128, 24], F32)
                    nc.vector.reduce_sum(
                        zred[:], zparts[:].rearrange("p (m c) -> p m c", c=3),
                        axis=mybir.AxisListType.X)
                    zredS = pm.tile([128, 24], F32)
                    nc.vector.tensor_scalar_mul(zredS[:], zred[:], scal[:, 2:3])
                    nc.sync.dma_start(z_in[:].rearrange("(m p) -> p m", p=128),
                                      zredS[:])
                    nc.gpsimd.collective_compute(
                        "AllReduce", OP.add,
                        replica_groups=[list(range(NCORES))],
                        ins=[z_in[:].opt()], outs=[z_out[:].opt()])
                    zmat_sb = pm.tile([T, K], F32)
                    nc.sync.dma_start(zmat_sb[:],
                                      z_out[0:TK].rearrange("(t k) -> t k", k=K))
                    nc.sync.dma_start(zmatOut[:], zmat_sb[:])
                    psZ = pmps.tile([128, 512], F32, name="psZ", tag="mps")
                    nc.tensor.matmul(psZ[:B, :K], onehot_sb[:], zmat_sb[:],
                                     start=True, stop=True)
                    rz = pm.tile([B, K], F32)
                    nc.vector.reciprocal(rz[:], psZ[:B, :K])
                    thz = pm.tile([B, K], F32)
                    nc.vector.tensor_mul(thz[:], theta[:], rz[:])
                    nc.sync.dma_start(thzOut[:], thz[:])
                    thzS = pm.tile([128, 128], BF16)
                    nc.vector.memset(thzS[:], 0.0)
                    nc.vector.tensor_scalar_mul(thzS[:B, :K], thz[:],
                                                scal[:, 3:4])
                    psT = pmps.tile([128, 512], BF16, name="psT", tag="mps")
                    nc.tensor.transpose(psT[:, :128], thzS[:], ident[:])
                    thzST = pm.tile([K, B], BF16)
                    nc.vector.tensor_copy(thzST[:], psT[:K, :128])
                    for m in range(24):
                        psF = pmps.tile([128, 512], F32, name="psF", tag="mps")
                        nc.tensor.matmul(psF[:, :B], RT_sb[:, m, :], thzST[:],
                                         start=True, stop=True)
                        nc.vector.tensor_mul(G2[:, m, :], psF[:, :B],
                                             F2_sb[:, m, :])

            # ---------------- Phase 3: mix matmuls + nll -----------------
            with tc.tile_pool(name="p3", bufs=1) as p3, \
                 tc.tile_pool(name="p3lm", bufs=2) as p3lm, \
                 tc.tile_pool(name="p3ps", bufs=1, space="PSUM") as p3ps:
                psd = [p3ps.tile([128, 512], F32, name=f"psd{i}")
                       for i in range(8)]
                m0 = 0
                for m, mrows in enumerate(MCH):
                    for n in range(8):
                        w = NW[n]
                        nc.tensor.matmul(psd[n][:, :w], G2[:mrows, m, :],
                                         exs[:mrows, m, n * 512:n * 512 + w],
                                         start=(m == 0), stop=(m == 23))
                    m0 += mrows
                for n in range(8):
                    w = NW[n]
                    lm = p3lm.tile([B, 512], F32, name="lm")
                    nc.scalar.activation(lm[:, :w], psd[n][:, :w], AF.Ln,
                                         bias=scal[:, 1:2])
                    junk = p3lm.tile([B, 512], F32, name="junk")
                    nc.vector.scalar_tensor_tensor(
                        junk[:, :w], lm[:, :w], 1.0,
                        bows_sb[:, n * 512:n * 512 + w],
                        op0=OP.bypass, op1=OP.mult,
                        accum_out=nllp[:, n:n + 1])
                nsum = p3.tile([B, 1], F32)
                nc.vector.reduce_sum(nsum[:], nllp[:], axis=mybir.AxisListType.X,
                                     negate=True)
                nc.sync.dma_start(nllOut[:], nsum[:])

    nc.compile()
    return nc


# ---------------------------------------------------------------------------
# host-side small sequential chains (fp32 numpy)
# ---------------------------------------------------------------------------

def _sigmoid(x):
    with np.errstate(over="ignore"):
        return (1.0 / (1.0 + np.exp(-x))).astype(np.float32)


def _kl_np(qm, qls, pm, pls):
    return 0.5 * np.sum(
        (np.exp(qls) + (qm - pm) ** 2) / (np.exp(pls) + 1e-6)
        - 1.0 + pls - qls, axis=-1, dtype=np.float32)


def _host_chains(inp):
    f = np.float32
    mu_a = np.asarray(inp["mu_q_alpha"], f).transpose(1, 0, 2)
    ls_a = np.asarray(inp["logsigma_q_alpha"], f).transpose(1, 0, 2)
    eps_a = np.asarray(inp["eps_alpha"], f)
    logdelta = f(np.log(f(DELTA)))
    alphas = (mu_a + eps_a * np.exp(0.5 * ls_a)).astype(f)
    kl_alpha = f(_kl_np(mu_a[0], ls_a[0], f(0.0), f(0.0)).sum()
                 + _kl_np(mu_a[1:], ls_a[1:], alphas[:-1], logdelta).sum())

    rnn_inp = np.asarray(inp["rnn_inp"], f)
    Wmap = np.asarray(inp["Wmap"], f)
    bmap = np.asarray(inp["bmap"], f)
    out = (rnn_inp @ Wmap.T + bmap).astype(f)
    Wih = np.asarray(inp["lstm_Wih"], f)
    Whh = np.asarray(inp["lstm_Whh"], f)
    bih = np.asarray(inp["lstm_bih"], f)
    bhh = np.asarray(inp["lstm_bhh"], f)
    for l in range(L):
        h = np.zeros(H, f)
        c = np.zeros(H, f)
        pre = (out @ Wih[l].T + (bih[l] + bhh[l])).astype(f)
        ys = np.empty((T, H), f)
        for t in range(T):
            g = pre[t] + Whh[l] @ h
            i_, f_, g_, o_ = np.split(g, 4)
            c = _sigmoid(f_) * c + _sigmoid(i_) * np.tanh(g_)
            h = (_sigmoid(o_) * np.tanh(c)).astype(f)
            ys[t] = h
        out = ys
    Wmu_e = np.asarray(inp["Wmu_e"], f)
    bmu_e = np.asarray(inp["bmu_e"], f)
    Wls_e = np.asarray(inp["Wls_e"], f)
    bls_e = np.asarray(inp["bls_e"], f)
    eps_eta = np.asarray(inp["eps_eta"], f)
    inp0 = np.concatenate([out[0], np.zeros(K, f)])
    mu0 = Wmu_e @ inp0 + bmu_e
    ls0 = Wls_e @ inp0 + bls_e
    eta = mu0 + eps_eta[0] * np.exp(0.5 * ls0)
    kl_eta = _kl_np(mu0, ls0, f(0.0), f(0.0))
    etas = np.empty((T, K), f)
    etas[0] = eta
    for t in range(1, T):
        it = np.concatenate([out[t], eta])
        mu_t = Wmu_e @ it + bmu_e
        ls_t = Wls_e @ it + bls_e
        eta = (mu_t + eps_eta[t] * np.exp(0.5 * ls_t)).astype(f)
        kl_eta = kl_eta + _kl_np(mu_t, ls_t, etas[t - 1], logdelta)
        etas[t] = eta
    return alphas, f(kl_alpha), etas, f(kl_eta)


def kernel(**inputs):
    f = np.float32
    bf = ml_dtypes.bfloat16
    if "nc" not in _CACHE:
        _CACHE["nc"] = _build_program()
    nc = _CACHE["nc"]

    bows = np.asarray(inputs["bows"], f)
    nb = np.asarray(inputs["normalized_bows"], f)
    times = np.asarray(inputs["times"]).astype(np.int64)
    num_docs = float(np.asarray(inputs["num_docs"]))
    W1 = np.asarray(inputs["W1"], f)
    b1 = np.asarray(inputs["b1"], f)
    W2 = np.asarray(inputs["W2"], f)
    b2 = np.asarray(inputs["b2"], f)
    Wmu_t = np.asarray(inputs["Wmu_t"], f)
    bmu_t = np.asarray(inputs["bmu_t"], f)
    Wls_t = np.asarray(inputs["Wls_t"], f)
    bls_t = np.asarray(inputs["bls_t"], f)
    rho = np.asarray(inputs["rho"], f)
    eps_theta = np.asarray(inputs["eps_theta"], f)

    alphas, kl_alpha, etas, kl_eta = _host_chains(inputs)
    eta_td = etas[times]                                   # [B, K]
    etaC = (eta_td @ W1[:, V:].T + b1).astype(f)           # [B, TH]

    # fp8 scaling: sampled logit max -> global scale s; q for the G2 side
    alf = np.ascontiguousarray(alphas.reshape(TK, E))
    samp = np.linspace(0, V - 1, 512).astype(np.int64)
    logit_s = alf @ rho[samp].T                            # [TK, 512]
    gmax = float(logit_s.max())
    lns = gmax - 6.0
    s = np.exp(np.float64(lns))
    zest = np.exp(logit_s.astype(np.float64) - lns).mean(axis=1) * V  # ~Z/s
    zmin_est = max(float(zest.min()) * s, 1e-30)
    q = int(np.clip(np.floor(np.log2(32.0 * zmin_est / s)), -30, 40))
    scal4 = np.zeros((128, 4), f)
    scal4[:, 0] = f(-lns)
    scal4[:, 1] = f(1e-6 * (2.0 ** q))
    scal4[:, 2] = f(s)
    scal4[:, 3] = f(s * (2.0 ** q))

    onehotT = (times[None, :] == np.arange(T)[:, None]).astype(f)
    rows = np.arange(24 * 128)
    tgrid = rows // 50                                     # time of row (>=60 pad)
    kgrid = rows % 50
    F2m = (times[None, :] == tgrid[:, None]).astype(bf)    # [3072, B]
    F2m = np.ascontiguousarray(
        F2m.reshape(24, 128, B).transpose(1, 0, 2).reshape(128, 24 * B))
    RTm = (np.arange(50)[:, None] == kgrid[None, :]).astype(bf)  # [50, 3072]
    RTm = np.ascontiguousarray(
        RTm.reshape(50, 24, 128).reshape(50, 24 * 128))

    alphasT = np.ascontiguousarray(alf.T.astype(bf))       # [E, TK]
    w2 = np.ascontiguousarray(W2.T).astype(bf)             # [TH(i), TH(o)]
    w2T = np.zeros((128, 7, TH), bf)
    for j in range(7):
        w2T[:KJ[j], j, :] = w2[j * 128:j * 128 + KJ[j], :]
    wmuls = np.concatenate([Wmu_t, Wls_t], axis=0).T.astype(bf)  # [TH, 2K]
    wmulsT = np.zeros((128, 7, 2 * K), bf)
    for j in range(7):
        wmulsT[:KJ[j], j, :] = wmuls[j * 128:j * 128 + KJ[j], :]
    b2R = np.zeros((128, 7), f)
    for j in range(7):
        b2R[:KJ[j], j] = b2[j * 128:j * 128 + KJ[j]]
    bmulsB = np.ascontiguousarray(
        np.broadcast_to(np.concatenate([bmu_t, bls_t]).astype(f), (B, 2 * K)))

    in_maps = []
    for c in range(NCORES):
        sl = slice(c * VS, (c + 1) * VS)
        in_maps.append({
            "nbT": np.ascontiguousarray(nb[:, sl].T).astype(bf),
            "w1vT": np.ascontiguousarray(W1[:, sl].T).astype(bf),
            "rhoT": np.ascontiguousarray(rho[sl, :].T).astype(bf),
            "alphasT": alphasT,
            "bowsS": np.ascontiguousarray(bows[:, sl]).astype(bf),
            "etaC": etaC,
            "etaTD": np.ascontiguousarray(eta_td.astype(f)),
            "epsTH": eps_theta,
            "onehotT": onehotT,
            "F2m": F2m,
            "RTm": RTm,
            "w2T": w2T,
            "wmulsT": wmulsT,
            "b2R": b2R,
            "bmulsB": bmulsB,
            "scal4": scal4,
        })

    global _LAST_IN_MAPS
    _LAST_IN_MAPS = in_maps
    res = bass_utils.run_bass_kernel_spmd(nc, in_maps,
                                          core_ids=list(range(NCORES)))
    _CACHE["res"] = res
    coeff = f(num_docs / B)
    nll_raw = sum(r["nllOut"].sum(dtype=np.float64) for r in res.results)
    # device loglik' = loglik_true + q ln2  =>  nll_true = nll_dev + q ln2 * sum(bows)
    nll_tot = f((nll_raw + q * np.log(2.0) * bows.sum(dtype=np.float64)) * coeff)
    klth_tot = f(res.results[0]["klthOut"].sum(dtype=np.float64) * coeff)
    nelbo = f(nll_tot + kl_alpha + kl_eta + klth_tot)
    return np.array([nelbo, nll_tot, kl_alpha, kl_eta, klth_tot], dtype=f)


# revision 27
# speedup vs baseline: 2.0688x; 1.2530x over previous
"""DETM nelbo kernel for 8 Trainium2 NeuronCores.

Sharding: vocabulary V=30000 split 8 ways (3750/core).
 - theta-MLP first layer contracts over V  -> per-core partial + AllReduce
   (computed transposed: h1preT [TH, B], so no on-chip transposes needed).
 - beta path: logit[t,k,v] = alphas.rho per V-slice; explog = exp(logit-ln s)
   is kept fully SBUF-resident in fp8e5 (e5m2). The softmax denominator
   Z[t,k] = s * sum_v exp(logit - ln s) is accumulated in f32 alongside the
   Exp activation and AllReduced.
 - nll: mix[b,v] = 2^q * sum_k theta[b,k]/Z[t_b,k] * explog[t_b,k,v] via
   24 x [128-row-chunk] fp8 matmuls: weights G2[m][p,b] =
   thz[b, p%50] * s * 2^q * [times[b] == row//50], built on-device from a
   tiny mask matmul; psum accumulates over all chunks.  loglik =
   Ln(psum + 1e-6*2^q) - q ln2; the -q ln2 * sum(bows) correction is applied
   on the host.
Small sequential chains (alpha reparam + KLs, LSTM, eta chain) are replicated
on the host in fp32 numpy (they are O(T*K*E) / O(T*H^2), ~1e-3 of the FLOPs).
"""
import sys

if "/opt/trn_rl_repo" not in sys.path:
    sys.path.insert(0, "/opt/trn_rl_repo")

import numpy as np
import ml_dtypes

import concourse.bass as bass
import concourse.mybir as mybir
import concourse.tile as tile
from concourse import bacc, bass_utils
from concourse.masks import make_identity

F32 = mybir.dt.float32
BF16 = mybir.dt.bfloat16
FP8 = mybir.dt.float8e5
AF = mybir.ActivationFunctionType
OP = mybir.AluOpType

V, K, E, T, B = 30000, 50, 300, 60, 128
TH, H, L = 800, 200, 3
NCORES = 8
VS = V // NCORES          # 3750
TK = T * K                # 3000
DELTA = 0.005

NCH = [(0, 1536), (1536, 1536), (3072, 678)]   # einsum column chunks
EK = [128, 128, 44]                            # E=300 contraction chunks
KJ = [128, 128, 128, 128, 128, 128, 32]        # TH=800 chunks
MCH = [128] * 23 + [56]                        # TK=3000 row chunks
P1CH = [128] * 29 + [38]                       # VS=3750 row chunks
NW = [512] * 7 + [166]                         # VS mix column chunks
VSP = 3840                                     # exs free-dim padded (%16 == 0)

_CACHE = {}


def _build_program():
    nc = bacc.Bacc("TRN2", target_bir_lowering=False, debug=False,
                   num_devices=NCORES)

    def din(name, shape, dt=F32):
        return nc.dram_tensor(name, shape, dt, kind="ExternalInput").ap()

    nbT = din("nbT", [VS, B], BF16)            # normalized_bows shard^T
    w1vT = din("w1vT", [VS, TH], BF16)         # W1[:, vshard].T
    rhoT = din("rhoT", [E, VS], BF16)          # rho[vshard].T
    alphasT = din("alphasT", [E, TK], BF16)    # alphas as [E, T*K]
    bowsS = din("bowsS", [B, VS], BF16)
    etaC = din("etaC", [B, TH])                # eta_td @ W1[:,V:].T + b1
    etaTD = din("etaTD", [B, K])
    epsTH = din("epsTH", [B, K])
    onehotT = din("onehotT", [T, B])           # f32, for Z gather matmul
    onehotA31 = din("onehotA31", [31, B])      # times 0..29 + catchall row
    F2m = din("F2m", [128, 24 * B], BF16)      # time mask by (row, chunk, doc)
    RTm = din("RTm", [50, 24 * 128], BF16)     # k-cyclic selection matrix
    w2T = din("w2T", [128, 7, TH], BF16)       # W2.T chunked [i-part, ichunk, o]
    wmulsT = din("wmulsT", [128, 7, 2 * K], BF16)
    b2R = din("b2R", [128, 7])
    bmulsB = din("bmulsB", [B, 2 * K])
    scal4 = din("scal4", [128, 4])             # cols: -ln s, 1e-6*2^q, s, s*2^q

    nllOut = nc.dram_tensor("nllOut", [B, 1], F32, kind="ExternalOutput").ap()
    klthOut = nc.dram_tensor("klthOut", [B, 1], F32, kind="ExternalOutput").ap()
    zmatOut = nc.dram_tensor("zmatOut", [T, K], F32, kind="ExternalOutput").ap()
    thzOut = nc.dram_tensor("thzOut", [B, K], F32, kind="ExternalOutput").ap()

    RKL = np.float32(0.5 / (1.0 + 1e-6))

    with tile.TileContext(nc) as tc:
        from contextlib import ExitStack
        with tc.tile_pool(name="outer", bufs=1) as outer, \
             tc.tile_pool(name="dramp", bufs=1, space="DRAM") as dram:
            scal = outer.tile([128, 4], F32)
            nc.sync.dma_start(scal[:], scal4[:])
            rho_sb = outer.tile([128, 3, VS], BF16)
            for kc in range(3):
                nc.sync.dma_start(rho_sb[:EK[kc], kc, :],
                                  rhoT[kc * 128:kc * 128 + EK[kc], :])
            onehot_sb = outer.tile([T, B], F32)
            nc.sync.dma_start(onehot_sb[:], onehotT[:])
            onehotA_sb = outer.tile([31, B], F32)
            nc.sync.dma_start(onehotA_sb[:], onehotA31[:])
            F2_sb = outer.tile([128, 24, B], BF16)
            nc.sync.dma_start(F2_sb[:], F2m[:].rearrange("p (m b) -> p m b", b=B))
            RT_sb = outer.tile([50, 24, 128], BF16)
            nc.sync.dma_start(RT_sb[:], RTm[:].rearrange("k (m p) -> k m p", p=128))
            etaC_sb = outer.tile([B, TH], F32)
            nc.sync.dma_start(etaC_sb[:], etaC[:])
            ident = outer.tile([128, 128], BF16)
            make_identity(nc, ident[:])

            exs = outer.tile([128, 24, VSP], FP8)     # resident explog/s
            nc.vector.memset(exs[:, 23, :], 0.0)      # pad rows of last chunk
            G2 = outer.tile([128, 24, B], FP8)
            zparts = outer.tile([128, 72], F32)
            nc.vector.memset(zparts[:], 0.0)
            nllp = outer.tile([B, 8], F32)
            nc.vector.memset(nllp[:], 0.0)
            bows_sb = outer.tile([B, VS], BF16)
            nc.sync.dma_start(bows_sb[:], bowsS[:])

            ar1_in = dram.tile([B, TH], F32)
            ar1_out = dram.tile([B, TH], F32, addr_space="Shared")
            z_inA = dram.tile([1536], F32)
            z_outA = dram.tile([1536], F32, addr_space="Shared")
            z_inB = dram.tile([1536], F32)
            z_outB = dram.tile([1536], F32, addr_space="Shared")

            groups = [list(range(NCORES))]
            with tc.tile_pool(name="peps", bufs=2, space="PSUM") as peps, \
                 tc.tile_pool(name="pa", bufs=3) as pa:
                es1 = ExitStack()
                p1in = es1.enter_context(tc.tile_pool(name="p1in", bufs=6))
                p1s = es1.enter_context(tc.tile_pool(name="p1s", bufs=1))
                p1ps = es1.enter_context(
                    tc.tile_pool(name="p1ps", bufs=1, space="PSUM"))
                psA = p1ps.tile([128, 2, 512], F32)
                p1_state = [0, 0]
                es2 = ExitStack()
                pm = None
                pmps = None

                def emit_p1(count):
                    for _ in range(count):
                        c, r0 = p1_state
                        if c >= 30:
                            return
                        rs = P1CH[c]
                        nb_t = p1in.tile([128, B], BF16, name="nb_t")
                        nc.sync.dma_start(nb_t[:rs], nbT[r0:r0 + rs, :])
                        w1_t = p1in.tile([128, TH], BF16, name="w1_t")
                        nc.sync.dma_start(w1_t[:rs], w1vT[r0:r0 + rs, :])
                        nc.tensor.matmul(psA[:, 0, :400], nb_t[:rs, :],
                                         w1_t[:rs, :400],
                                         start=(c == 0), stop=(c == 29))
                        nc.tensor.matmul(psA[:, 1, :400], nb_t[:rs, :],
                                         w1_t[:rs, 400:],
                                         start=(c == 0), stop=(c == 29))
                        p1_state[0] = c + 1
                        p1_state[1] = r0 + rs

                for m, mrows in enumerate(MCH):
                    m0 = m * 128
                    a_t = pa.tile([128, 3, 128], BF16, name="a_t")
                    for kc in range(3):
                        nc.sync.dma_start(
                            a_t[:EK[kc], kc, :mrows],
                            alphasT[kc * 128:kc * 128 + EK[kc], m0:m0 + mrows])
                    for nch, (n0, nw) in enumerate(NCH):
                        ps = peps.tile([128, 1536], F32, name="psE")
                        nsub = (nw + 511) // 512
                        for s in range(nsub):
                            w = min(512, nw - s * 512)
                            for kc in range(3):
                                nc.tensor.matmul(
                                    ps[:mrows, s * 512:s * 512 + w],
                                    a_t[:EK[kc], kc, :mrows],
                                    rho_sb[:EK[kc], kc,
                                           n0 + s * 512:n0 + s * 512 + w],
                                    start=(kc == 0), stop=(kc == 2))
                        nc.scalar.activation(
                            exs[:mrows, m, n0:n0 + nw], ps[:mrows, :nw],
                            AF.Exp, bias=scal[:mrows, 0:1],
                            accum_out=zparts[:mrows,
                                             m * 3 + nch:m * 3 + nch + 1])

                    if m >= 2:
                        emit_p1(6)
                    if m == 6:
                        h1preS = p1s.tile([B, TH], F32)
                        nc.vector.tensor_copy(h1preS[:], psA[:, :, :400])
                        nc.sync.dma_start(ar1_in[:], h1preS[:])
                        es1.close()
                        nc.gpsimd.collective_compute(
                            "AllReduce", OP.add, replica_groups=groups,
                            ins=[ar1_in[:].opt()], outs=[ar1_out[:].opt()])
                        pm = es2.enter_context(tc.tile_pool(name="pm", bufs=1))
                        pmps = es2.enter_context(
                            tc.tile_pool(name="pmps", bufs=2, space="PSUM"))
                    if m == 7:
                        # ---- MLP (executes when AR1 lands) ----
                        h1pre = pm.tile([B, TH], F32)
                        nc.sync.dma_start(h1pre[:], ar1_out[:])
                        hsum = pm.tile([B, TH], F32)
                        nc.vector.tensor_add(hsum[:], h1pre[:], etaC_sb[:])
                        h1b = pm.tile([B, TH], BF16)
                        nc.scalar.activation(h1b[:], hsum[:], AF.Relu)
                        h1T = pm.tile([128, 7, B], BF16)
                        for j in range(7):
                            ptp = pmps.tile([128, 512], BF16, name="ptp",
                                            tag="mps")
                            nc.tensor.transpose(
                                ptp[:KJ[j], :128],
                                h1b[:, j * 128:j * 128 + KJ[j]], ident[:])
                            nc.vector.tensor_copy(h1T[:KJ[j], j, :],
                                                  ptp[:KJ[j], :128])
                        w2T_sb = pm.tile([128, 7, TH], BF16)
                        for j in range(7):
                            nc.sync.dma_start(w2T_sb[:KJ[j], j, :],
                                              w2T[:KJ[j], j, :])
                        b2R_sb = pm.tile([128, 7], F32)
                        nc.sync.dma_start(b2R_sb[:], b2R[:])
                        h2T = pm.tile([128, 7, B], BF16)
                        for jo in range(7):
                            wjo = KJ[jo]
                            psH = pmps.tile([128, 512], F32, name="psH",
                                            tag="mps")
                            for ji in range(7):
                                nc.tensor.matmul(
                                    psH[:wjo, :B],
                                    w2T_sb[:KJ[ji], ji,
                                           jo * 128:jo * 128 + wjo],
                                    h1T[:KJ[ji], ji, :],
                                    start=(ji == 0), stop=(ji == 6))
                            nc.scalar.activation(h2T[:wjo, jo, :],
                                                 psH[:wjo, :B], AF.Relu,
                                                 bias=b2R_sb[:wjo, jo:jo + 1])
                        wmuls_sb = pm.tile([128, 7, 2 * K], BF16)
                        nc.sync.dma_start(wmuls_sb[:], wmulsT[:])
                        psM = pmps.tile([128, 512], F32, name="psM", tag="mps")
                        for ji in range(7):
                            nc.tensor.matmul(psM[:B, :2 * K],
                                             h2T[:KJ[ji], ji, :],
                                             wmuls_sb[:KJ[ji], ji, :],
                                             start=(ji == 0), stop=(ji == 6))
                        bmuls_sb = pm.tile([B, 2 * K], F32)
                        nc.sync.dma_start(bmuls_sb[:], bmulsB[:])
                        muls = pm.tile([B, 2 * K], F32)
                        nc.vector.tensor_add(muls[:], psM[:B, :2 * K],
                                             bmuls_sb[:])
                        mu = muls[:, :K]
                        ls = muls[:, K:]
                        sd = pm.tile([B, K], F32)
                        nc.scalar.activation(sd[:], ls, AF.Exp, scale=0.5)
                        epsTH_sb = pm.tile([B, K], F32)
                        nc.sync.dma_start(epsTH_sb[:], epsTH[:])
                        ez0 = pm.tile([B, K], F32)
                        nc.vector.tensor_mul(ez0[:], epsTH_sb[:], sd[:])
                        zt = pm.tile([B, K], F32)
                        nc.vector.tensor_add(zt[:], mu, ez0[:])
                        zm = pm.tile([B, 1], F32)
                        nc.vector.reduce_max(zm[:], zt[:],
                                             axis=mybir.AxisListType.X,
                                             negate=True)
                        et = pm.tile([B, K], F32)
                        se = pm.tile([B, 1], F32)
                        nc.scalar.activation(et[:], zt[:], AF.Exp, bias=zm[:],
                                             accum_out=se[:])
                        rse = pm.tile([B, 1], F32)
                        nc.vector.reciprocal(rse[:], se[:])
                        theta = pm.tile([B, K], F32)
                        nc.vector.tensor_scalar_mul(theta[:], et[:], rse[:])
                        # kl_theta
                        etaTD_sb = pm.tile([B, K], F32)
                        nc.sync.dma_start(etaTD_sb[:], etaTD[:])
                        sd2 = pm.tile([B, K], F32)
                        nc.vector.tensor_mul(sd2[:], sd[:], sd[:])
                        dd = pm.tile([B, K], F32)
                        nc.vector.tensor_sub(dd[:], mu, etaTD_sb[:])
                        dd2 = pm.tile([B, K], F32)
                        nc.vector.tensor_mul(dd2[:], dd[:], dd[:])
                        uu = pm.tile([B, K], F32)
                        sA = pm.tile([B, 1], F32)
                        nc.vector.scalar_tensor_tensor(
                            uu[:], dd2[:], 1.0, sd2[:],
                            op0=OP.bypass, op1=OP.add, accum_out=sA[:])
                        sB_ = pm.tile([B, 1], F32)
                        nc.vector.reduce_sum(sB_[:], ls,
                                             axis=mybir.AxisListType.X)
                        q1 = pm.tile([B, 1], F32)
                        nc.vector.tensor_scalar(q1[:], sA[:], float(RKL),
                                                -float(K) * 0.5,
                                                op0=OP.mult, op1=OP.add)
                        q2 = pm.tile([B, 1], F32)
                        nc.vector.tensor_scalar_mul(q2[:], sB_[:], 0.5)
                        klth = pm.tile([B, 1], F32)
                        nc.vector.tensor_sub(klth[:], q1[:], q2[:])
                        nc.sync.dma_start(klthOut[:], klth[:])
                    if m == 12:
                        # fire Z AllReduce for chunks 0..11 (z rows 0..1535)
                        zredA = pm.tile([128, 12], F32)
                        nc.vector.reduce_sum(
                            zredA[:],
                            zparts[:, 0:36].rearrange("p (m c) -> p m c", c=3),
                            axis=mybir.AxisListType.X)
                        zredSA = pm.tile([128, 12], F32)
                        nc.vector.tensor_scalar_mul(zredSA[:], zredA[:],
                                                    scal[:, 2:3])
                        nc.sync.dma_start(
                            z_inA[:].rearrange("(m p) -> p m", p=128),
                            zredSA[:])
                        nc.gpsimd.collective_compute(
                            "AllReduce", OP.add, replica_groups=groups,
                            ins=[z_inA[:].opt()], outs=[z_outA[:].opt()])

                # ---- Z second half (chunks 12..23, z rows 1536..3071) ----
                zredB = pm.tile([128, 12], F32)
                nc.vector.reduce_sum(
                    zredB[:],
                    zparts[:, 36:72].rearrange("p (m c) -> p m c", c=3),
                    axis=mybir.AxisListType.X)
                zredSB = pm.tile([128, 12], F32)
                nc.vector.tensor_scalar_mul(zredSB[:], zredB[:], scal[:, 2:3])
                nc.sync.dma_start(z_inB[:].rearrange("(m p) -> p m", p=128),
                                  zredSB[:])
                nc.gpsimd.collective_compute(
                    "AllReduce", OP.add, replica_groups=groups,
                    ins=[z_inB[:].opt()], outs=[z_outB[:].opt()])

                # ---- thz-A (times 0..29 exact; others -> Z=1) + G2 0..10 --
                zmatA = pm.tile([31, K], F32)
                nc.vector.memset(zmatA[:], 1.0)
                nc.sync.dma_start(zmatA[0:30, :],
                                  z_outA[0:1500].rearrange("(t k) -> t k", k=K))
                psZA = pmps.tile([128, 512], F32, name="psZ", tag="mps")
                nc.tensor.matmul(psZA[:B, :K], onehotA_sb[:], zmatA[:],
                                 start=True, stop=True)
                rzA = pm.tile([B, K], F32)
                nc.vector.reciprocal(rzA[:], psZA[:B, :K])
                thzSA = pm.tile([128, 128], BF16)
                nc.vector.memset(thzSA[:], 0.0)
                thzA = pm.tile([B, K], F32)
                nc.vector.tensor_mul(thzA[:], theta[:], rzA[:])
                nc.vector.tensor_scalar_mul(thzSA[:B, :K], thzA[:],
                                            scal[:, 3:4])
                psTA = pmps.tile([128, 512], BF16, name="psT", tag="mps")
                nc.tensor.transpose(psTA[:, :128], thzSA[:], ident[:])
                thzSTA = pm.tile([K, B], BF16)
                nc.vector.tensor_copy(thzSTA[:], psTA[:K, :128])
                for m2 in range(11):
                    psF = pmps.tile([128, 512], F32, name="psF", tag="mps")
                    nc.tensor.matmul(psF[:, :B], RT_sb[:, m2, :], thzSTA[:],
                                     start=True, stop=True)
                    nc.vector.tensor_mul(G2[:, m2, :], psF[:, :B],
                                         F2_sb[:, m2, :])

                # ---- full zmat + thz + G2 11..23 ----
                zmatF = pm.tile([T, K], F32)
                nc.sync.dma_start(zmatF[0:30, :],
                                  z_outA[0:1500].rearrange("(t k) -> t k", k=K))
                nc.sync.dma_start(
                    zmatF[30:31, 0:36],
                    z_outA[1500:1536].rearrange("(t k) -> t k", k=36))
                nc.sync.dma_start(
                    zmatF[30:31, 36:50],
                    z_outB[0:14].rearrange("(t k) -> t k", k=14))
                nc.sync.dma_start(
                    zmatF[31:60, :],
                    z_outB[14:1464].rearrange("(t k) -> t k", k=K))
                nc.sync.dma_start(zmatOut[:], zmatF[:])
                psZF = pmps.tile([128, 512], F32, name="psZ", tag="mps")
                nc.tensor.matmul(psZF[:B, :K], onehot_sb[:], zmatF[:],
                                 start=True, stop=True)
                rz = pm.tile([B, K], F32)
                nc.vector.reciprocal(rz[:], psZF[:B, :K])
                thz = pm.tile([B, K], F32)
                nc.vector.tensor_mul(thz[:], theta[:], rz[:])
                nc.sync.dma_start(thzOut[:], thz[:])
                thzS = pm.tile([128, 128], BF16)
                nc.vector.memset(thzS[:], 0.0)
                nc.vector.tensor_scalar_mul(thzS[:B, :K], thz[:], scal[:, 3:4])
                psT2 = pmps.tile([128, 512], BF16, name="psT", tag="mps")
                nc.tensor.transpose(psT2[:, :128], thzS[:], ident[:])
                thzST = pm.tile([K, B], BF16)
                nc.vector.tensor_copy(thzST[:], psT2[:K, :128])
                for m2 in range(11, 24):
                    psF = pmps.tile([128, 512], F32, name="psF", tag="mps")
                    nc.tensor.matmul(psF[:, :B], RT_sb[:, m2, :], thzST[:],
                                     start=True, stop=True)
                    nc.vector.tensor_mul(G2[:, m2, :], psF[:, :B],
                                         F2_sb[:, m2, :])
                es2.close()

            # ---------------- Phase 3: mix matmuls + nll -----------------
            with tc.tile_pool(name="p3", bufs=1) as p3, \
                 tc.tile_pool(name="p3lm", bufs=2) as p3lm, \
                 tc.tile_pool(name="p3ps", bufs=1, space="PSUM") as p3ps:
                psd = [p3ps.tile([128, 512], F32, name=f"psd{i}")
                       for i in range(8)]
                DR = mybir.MatmulPerfMode.DoubleRow

                def mix_pair(mm, start, stop):
                    for n in range(8):
                        w = NW[n]
                        nc.tensor.matmul(
                            psd[n][:, :w], G2[:, mm:mm + 2, :],
                            exs[:, mm:mm + 2, n * 512:n * 512 + w],
                            perf_mode=DR, start=start, stop=stop)

                def mix_single(mm, start, stop):
                    mrows = MCH[mm]
                    for n in range(8):
                        w = NW[n]
                        nc.tensor.matmul(
                            psd[n][:, :w], G2[:mrows, mm, :],
                            exs[:mrows, mm, n * 512:n * 512 + w],
                            start=start, stop=stop)

                for p in range(5):                 # chunks 0..9
                    mix_pair(2 * p, start=(p == 0), stop=False)
                mix_single(10, False, False)
                mix_single(11, False, False)
                for p in range(6, 12):             # chunks 12..23
                    mix_pair(2 * p, start=False, stop=(p == 11))

                for n in range(8):
                    w = NW[n]
                    lm = p3lm.tile([B, 512], F32, name="lm")
                    nc.scalar.activation(lm[:, :w], psd[n][:, :w], AF.Ln,
                                         bias=scal[:, 1:2])
                    junk = p3lm.tile([B, 512], F32, name="junk")
                    nc.vector.scalar_tensor_tensor(
                        junk[:, :w], lm[:, :w], 1.0,
                        bows_sb[:, n * 512:n * 512 + w],
                        op0=OP.bypass, op1=OP.mult,
                        accum_out=nllp[:, n:n + 1])
                nsum = p3.tile([B, 1], F32)
                nc.vector.reduce_sum(nsum[:], nllp[:],
                                     axis=mybir.AxisListType.X, negate=True)
                nc.sync.dma_start(nllOut[:], nsum[:])

    nc.compile()
    return nc


# ---------------------------------------------------------------------------
# host-side small sequential chains (fp32 numpy)
# ---------------------------------------------------------------------------

def _sigmoid(x):
    with np.errstate(over="ignore"):
        return (1.0 / (1.0 + np.exp(-x))).astype(np.float32)


def _kl_np(qm, qls, pm, pls):
    return 0.5 * np.sum(
        (np.exp(qls) + (qm - pm) ** 2) / (np.exp(pls) + 1e-6)
        - 1.0 + pls - qls, axis=-1, dtype=np.float32)


def _host_chains(inp):
    f = np.float32
    mu_a = np.asarray(inp["mu_q_alpha"], f).transpose(1, 0, 2)
    ls_a = np.asarray(inp["logsigma_q_alpha"], f).transpose(1, 0, 2)
    eps_a = np.asarray(inp["eps_alpha"], f)
    logdelta = f(np.log(f(DELTA)))
    alphas = (mu_a + eps_a * np.exp(0.5 * ls_a)).astype(f)
    kl_alpha = f(_kl_np(mu_a[0], ls_a[0], f(0.0), f(0.0)).sum()
                 + _kl_np(mu_a[1:], ls_a[1:], alphas[:-1], logdelta).sum())

    rnn_inp = np.asarray(inp["rnn_inp"], f)
    Wmap = np.asarray(inp["Wmap"], f)
    bmap = np.asarray(inp["bmap"], f)
    out = (rnn_inp @ Wmap.T + bmap).astype(f)
    Wih = np.asarray(inp["lstm_Wih"], f)
    Whh = np.asarray(inp["lstm_Whh"], f)
    bih = np.asarray(inp["lstm_bih"], f)
    bhh = np.asarray(inp["lstm_bhh"], f)
    for l in range(L):
        h = np.zeros(H, f)
        c = np.zeros(H, f)
        pre = (out @ Wih[l].T + (bih[l] + bhh[l])).astype(f)
        ys = np.empty((T, H), f)
        for t in range(T):
            g = pre[t] + Whh[l] @ h
            i_, f_, g_, o_ = np.split(g, 4)
            c = _sigmoid(f_) * c + _sigmoid(i_) * np.tanh(g_)
            h = (_sigmoid(o_) * np.tanh(c)).astype(f)
            ys[t] = h
        out = ys
    Wmu_e = np.asarray(inp["Wmu_e"], f)
    bmu_e = np.asarray(inp["bmu_e"], f)
    Wls_e = np.asarray(inp["Wls_e"], f)
    bls_e = np.asarray(inp["bls_e"], f)
    eps_eta = np.asarray(inp["eps_eta"], f)
    inp0 = np.concatenate([out[0], np.zeros(K, f)])
    mu0 = Wmu_e @ inp0 + bmu_e
    ls0 = Wls_e @ inp0 + bls_e
    eta = mu0 + eps_eta[0] * np.exp(0.5 * ls0)
    kl_eta = _kl_np(mu0, ls0, f(0.0), f(0.0))
    etas = np.empty((T, K), f)
    etas[0] = eta
    for t in range(1, T):
        it = np.concatenate([out[t], eta])
        mu_t = Wmu_e @ it + bmu_e
        ls_t = Wls_e @ it + bls_e
        eta = (mu_t + eps_eta[t] * np.exp(0.5 * ls_t)).astype(f)
        kl_eta = kl_eta + _kl_np(mu_t, ls_t, etas[t - 1], logdelta)
        etas[t] = eta
    return alphas, f(kl_alpha), etas, f(kl_eta)


def kernel(**inputs):
    f = np.float32
    bf = ml_dtypes.bfloat16
    if "nc" not in _CACHE:
        _CACHE["nc"] = _build_program()
    nc = _CACHE["nc"]

    bows = np.asarray(inputs["bows"], f)
    nb = np.asarray(inputs["normalized_bows"], f)
    times = np.asarray(inputs["times"]).astype(np.int64)
    num_docs = float(np.asarray(inputs["num_docs"]))
    W1 = np.asarray(inputs["W1"], f)
    b1 = np.asarray(inputs["b1"], f)
    W2 = np.asarray(inputs["W2"], f)
    b2 = np.asarray(inputs["b2"], f)
    Wmu_t = np.asarray(inputs["Wmu_t"], f)
    bmu_t = np.asarray(inputs["bmu_t"], f)
    Wls_t = np.asarray(inputs["Wls_t"], f)
    bls_t = np.asarray(inputs["bls_t"], f)
    rho = np.asarray(inputs["rho"], f)
    eps_theta = np.asarray(inputs["eps_theta"], f)

    alphas, kl_alpha, etas, kl_eta = _host_chains(inputs)
    eta_td = etas[times]                                   # [B, K]
    etaC = (eta_td @ W1[:, V:].T + b1).astype(f)           # [B, TH]

    # fp8 scaling: sampled logit max -> global scale s; q for the G2 side
    alf = np.ascontiguousarray(alphas.reshape(TK, E))
    samp = np.linspace(0, V - 1, 512).astype(np.int64)
    logit_s = alf @ rho[samp].T                            # [TK, 512]
    gmax = float(logit_s.max())
    lns = gmax - 6.0
    s = np.exp(np.float64(lns))
    zest = np.exp(logit_s.astype(np.float64) - lns).mean(axis=1) * V  # ~Z/s
    zmin_est = max(float(zest.min()) * s, 1e-30)
    q = int(np.clip(np.floor(np.log2(32.0 * zmin_est / s)), -30, 40))
    scal4 = np.zeros((128, 4), f)
    scal4[:, 0] = f(-lns)
    scal4[:, 1] = f(1e-6 * (2.0 ** q))
    scal4[:, 2] = f(s)
    scal4[:, 3] = f(s * (2.0 ** q))

    onehotT = (times[None, :] == np.arange(T)[:, None]).astype(f)
    onehotA31v = np.concatenate(
        [onehotT[:30], (times[None, :] >= 30).astype(f)], axis=0)
    rows = np.arange(24 * 128)
    tgrid = rows // 50                                     # time of row (>=60 pad)
    kgrid = rows % 50
    F2m = (times[None, :] == tgrid[:, None]).astype(bf)    # [3072, B]
    F2m = np.ascontiguousarray(
        F2m.reshape(24, 128, B).transpose(1, 0, 2).reshape(128, 24 * B))
    RTm = (np.arange(50)[:, None] == kgrid[None, :]).astype(bf)  # [50, 3072]
    RTm = np.ascontiguousarray(
        RTm.reshape(50, 24, 128).reshape(50, 24 * 128))

    alphasT = np.ascontiguousarray(alf.T.astype(bf))       # [E, TK]
    w2 = np.ascontiguousarray(W2.T).astype(bf)             # [TH(i), TH(o)]
    w2T = np.zeros((128, 7, TH), bf)
    for j in range(7):
        w2T[:KJ[j], j, :] = w2[j * 128:j * 128 + KJ[j], :]
    wmuls = np.concatenate([Wmu_t, Wls_t], axis=0).T.astype(bf)  # [TH, 2K]
    wmulsT = np.zeros((128, 7, 2 * K), bf)
    for j in range(7):
        wmulsT[:KJ[j], j, :] = wmuls[j * 128:j * 128 + KJ[j], :]
    b2R = np.zeros((128, 7), f)
    for j in range(7):
        b2R[:KJ[j], j] = b2[j * 128:j * 128 + KJ[j]]
    bmulsB = np.ascontiguousarray(
        np.broadcast_to(np.concatenate([bmu_t, bls_t]).astype(f), (B, 2 * K)))

    in_maps = []
    for c in range(NCORES):
        sl = slice(c * VS, (c + 1) * VS)
        in_maps.append({
            "nbT": np.ascontiguousarray(nb[:, sl].T).astype(bf),
            "w1vT": np.ascontiguousarray(W1[:, sl].T).astype(bf),
            "rhoT": np.ascontiguousarray(rho[sl, :].T).astype(bf),
            "alphasT": alphasT,
            "bowsS": np.ascontiguousarray(bows[:, sl]).astype(bf),
            "etaC": etaC,
            "etaTD": np.ascontiguousarray(eta_td.astype(f)),
            "epsTH": eps_theta,
            "onehotT": onehotT,
            "onehotA31": onehotA31v,
            "F2m": F2m,
            "RTm": RTm,
            "w2T": w2T,
            "wmulsT": wmulsT,
            "b2R": b2R,
            "bmulsB": bmulsB,
            "scal4": scal4,
        })

    global _LAST_IN_MAPS
    _LAST_IN_MAPS = in_maps
    res = bass_utils.run_bass_kernel_spmd(nc, in_maps,
                                          core_ids=list(range(NCORES)))
    _CACHE["res"] = res
    coeff = f(num_docs / B)
    nll_raw = sum(r["nllOut"].sum(dtype=np.float64) for r in res.results)
    # device loglik' = loglik_true + q ln2  =>  nll_true = nll_dev + q ln2 * sum(bows)
    nll_tot = f((nll_raw + q * np.log(2.0) * bows.sum(dtype=np.float64)) * coeff)
    klth_tot = f(res.results[0]["klthOut"].sum(dtype=np.float64) * coeff)
    nelbo = f(nll_tot + kl_alpha + kl_eta + klth_tot)
    return np.array([nelbo, nll_tot, kl_alpha, kl_eta, klth_tot], dtype=f)


# revision 33
# speedup vs baseline: 2.3504x; 1.1361x over previous
"""DETM nelbo kernel for 8 Trainium2 NeuronCores.

Sharding: vocabulary V=30000 split 8 ways (3750/core).
 - theta-MLP first layer contracts over V  -> per-core partial + AllReduce
   (computed transposed: h1preT [TH, B], so no on-chip transposes needed).
 - beta path: logit[t,k,v] = alphas.rho per V-slice; explog = exp(logit-ln s)
   is kept fully SBUF-resident in fp8e5 (e5m2). The softmax denominator
   Z[t,k] = s * sum_v exp(logit - ln s) is accumulated in f32 alongside the
   Exp activation and AllReduced.
 - nll: mix[b,v] = 2^q * sum_k theta[b,k]/Z[t_b,k] * explog[t_b,k,v] via
   24 x [128-row-chunk] fp8 matmuls: weights G2[m][p,b] =
   thz[b, p%50] * s * 2^q * [times[b] == row//50], built on-device from a
   tiny mask matmul; psum accumulates over all chunks.  loglik =
   Ln(psum + 1e-6*2^q) - q ln2; the -q ln2 * sum(bows) correction is applied
   on the host.
Small sequential chains (alpha reparam + KLs, LSTM, eta chain) are replicated
on the host in fp32 numpy (they are O(T*K*E) / O(T*H^2), ~1e-3 of the FLOPs).
"""
import sys

if "/opt/trn_rl_repo" not in sys.path:
    sys.path.insert(0, "/opt/trn_rl_repo")

import numpy as np
import ml_dtypes

import concourse.bass as bass
import concourse.mybir as mybir
import concourse.tile as tile
from concourse import bacc, bass_utils
from concourse.masks import make_identity

F32 = mybir.dt.float32
BF16 = mybir.dt.bfloat16
FP8 = mybir.dt.float8e5
AF = mybir.ActivationFunctionType
OP = mybir.AluOpType

V, K, E, T, B = 30000, 50, 300, 60, 128
TH, H, L = 800, 200, 3
NCORES = 8
VS = V // NCORES          # 3750
TK = T * K                # 3000
DELTA = 0.005

NCH = [(0, 1536), (1536, 1536), (3072, 678)]   # einsum column chunks
EK = [128, 128, 44]                            # E=300 contraction chunks
KJ = [128, 128, 128, 128, 128, 128, 32]        # TH=800 chunks
MCH = [128] * 23 + [56]                        # TK=3000 row chunks
P1CH = [128] * 29 + [38]                       # VS=3750 row chunks
NW = [512] * 7 + [166]                         # VS mix column chunks
VSP = 3840                                     # exs free-dim padded (%16 == 0)

_CACHE = {}


def _build_program():
    nc = bacc.Bacc("TRN2", target_bir_lowering=False, debug=False,
                   num_devices=NCORES)

    def din(name, shape, dt=F32):
        return nc.dram_tensor(name, shape, dt, kind="ExternalInput").ap()

    nbT = din("nbT", [VS, B], BF16)            # normalized_bows shard^T
    w1vT = din("w1vT", [VS, TH], BF16)         # W1[:, vshard].T
    rhoT = din("rhoT", [E, VS], FP8)           # rho[vshard].T
    alphasT = din("alphasT", [E, TK], FP8)     # alphas as [E, T*K]
    bowsS = din("bowsS", [B, VS], BF16)
    etaC = din("etaC", [B, TH])                # eta_td @ W1[:,V:].T + b1
    etaTD = din("etaTD", [B, K])
    epsTH = din("epsTH", [B, K])
    onehotT = din("onehotT", [T, B])           # f32, for Z gather matmul
    onehotA31 = din("onehotA31", [31, B])      # times 0..29 + catchall row
    F2m = din("F2m", [128, 24 * B], BF16)      # time mask by (row, chunk, doc)
    RTm = din("RTm", [50, 24 * 128], BF16)     # k-cyclic selection matrix
    w2T = din("w2T", [128, 7, TH], BF16)       # W2.T chunked [i-part, ichunk, o]
    wmulsT = din("wmulsT", [128, 7, 2 * K], BF16)
    b2R = din("b2R", [128, 7])
    bmulsB = din("bmulsB", [B, 2 * K])
    scal4 = din("scal4", [128, 4])             # cols: -ln s, 1e-6*2^q, s, s*2^q

    nllOut = nc.dram_tensor("nllOut", [B, 1], F32, kind="ExternalOutput").ap()
    klthOut = nc.dram_tensor("klthOut", [B, 1], F32, kind="ExternalOutput").ap()
    zmatOut = nc.dram_tensor("zmatOut", [T, K], F32, kind="ExternalOutput").ap()
    thzOut = nc.dram_tensor("thzOut", [B, K], F32, kind="ExternalOutput").ap()

    RKL = np.float32(0.5 / (1.0 + 1e-6))

    with tile.TileContext(nc) as tc:
        from contextlib import ExitStack
        with tc.tile_pool(name="outer", bufs=1) as outer, \
             tc.tile_pool(name="dramp", bufs=1, space="DRAM") as dram:
            scal = outer.tile([128, 4], F32)
            nc.sync.dma_start(scal[:], scal4[:])
            rho_sb = outer.tile([128, 3, VSP], FP8)
            for kc in range(3):
                nc.sync.dma_start(rho_sb[:EK[kc], kc, :VS],
                                  rhoT[kc * 128:kc * 128 + EK[kc], :])
            onehot_sb = outer.tile([T, B], F32)
            onehotA_sb = outer.tile([31, B], F32)
            F2_sb = outer.tile([128, 24, B], BF16)
            RT_sb = outer.tile([50, 24, 128], BF16)
            etaC_sb = outer.tile([B, TH], F32)
            ident = outer.tile([128, 128], BF16)
            make_identity(nc, ident[:])

            exs = outer.tile([128, 24, VSP], FP8)     # resident explog/s
            nc.vector.memset(exs[:, 23, :], 0.0)      # pad rows of last chunk
            G2 = outer.tile([128, 24, B], FP8)
            zparts = outer.tile([128, 72], F32)
            nc.vector.memset(zparts[:], 0.0)
            nllp = outer.tile([B, 8], F32)
            nc.vector.memset(nllp[:], 0.0)
            bows_sb = outer.tile([B, VS], BF16)

            ar1_in = dram.tile([B, TH], F32)
            ar1_out = dram.tile([B, TH], F32, addr_space="Shared")
            z_inA = dram.tile([1536], F32)
            z_outA = dram.tile([1536], F32, addr_space="Shared")
            z_inB = dram.tile([1536], F32)
            z_outB = dram.tile([1536], F32, addr_space="Shared")

            groups = [list(range(NCORES))]
            with tc.tile_pool(name="peps", bufs=2, space="PSUM") as peps, \
                 tc.tile_pool(name="pa", bufs=3) as pa:
                es1 = ExitStack()
                p1in = es1.enter_context(tc.tile_pool(name="p1in", bufs=6))
                p1s = es1.enter_context(tc.tile_pool(name="p1s", bufs=1))
                p1ps = es1.enter_context(
                    tc.tile_pool(name="p1ps", bufs=1, space="PSUM"))
                psA = p1ps.tile([128, 2, 512], F32)
                p1_state = [0, 0]
                es2 = ExitStack()
                pm = None
                pmps = None

                def emit_p1(count):
                    for _ in range(count):
                        c, r0 = p1_state
                        if c >= 30:
                            return
                        rs = P1CH[c]
                        nb_t = p1in.tile([128, B], BF16, name="nb_t")
                        nc.sync.dma_start(nb_t[:rs], nbT[r0:r0 + rs, :])
                        w1_t = p1in.tile([128, TH], BF16, name="w1_t")
                        nc.sync.dma_start(w1_t[:rs], w1vT[r0:r0 + rs, :])
                        nc.tensor.matmul(psA[:, 0, :400], nb_t[:rs, :],
                                         w1_t[:rs, :400],
                                         start=(c == 0), stop=(c == 29))
                        nc.tensor.matmul(psA[:, 1, :400], nb_t[:rs, :],
                                         w1_t[:rs, 400:],
                                         start=(c == 0), stop=(c == 29))
                        p1_state[0] = c + 1
                        p1_state[1] = r0 + rs

                DRmode = mybir.MatmulPerfMode.DoubleRow
                for m, mrows in enumerate(MCH):
                    m0 = m * 128
                    a_t = pa.tile([128, 3, 128], FP8, name="a_t")
                    for kc in range(3):
                        nc.sync.dma_start(
                            a_t[:EK[kc], kc, :mrows],
                            alphasT[kc * 128:kc * 128 + EK[kc], m0:m0 + mrows])
                    for nch, (n0, nw) in enumerate(NCH):
                        ps = peps.tile([128, 1536], F32, name="psE")
                        nsub = (nw + 511) // 512
                        for s in range(nsub):
                            w = min(512, nw - s * 512)
                            c0 = n0 + s * 512
                            nc.tensor.matmul(
                                ps[:mrows, s * 512:s * 512 + w],
                                a_t[:, 0:2, :mrows],
                                rho_sb[:, 0:2, c0:c0 + w],
                                perf_mode=DRmode, start=True, stop=False)
                            nc.tensor.matmul(
                                ps[:mrows, s * 512:s * 512 + w],
                                a_t[:EK[2], 2, :mrows],
                                rho_sb[:EK[2], 2, c0:c0 + w],
                                start=False, stop=True)
                        nc.scalar.activation(
                            exs[:mrows, m, n0:n0 + nw], ps[:mrows, :nw],
                            AF.Exp, bias=scal[:mrows, 0:1],
                            accum_out=zparts[:mrows,
                                             m * 3 + nch:m * 3 + nch + 1])

                    if m == 1:
                        nc.sync.dma_start(onehot_sb[:], onehotT[:])
                        nc.sync.dma_start(onehotA_sb[:], onehotA31[:])
                        nc.sync.dma_start(
                            F2_sb[:], F2m[:].rearrange("p (m b) -> p m b", b=B))
                        nc.sync.dma_start(
                            RT_sb[:],
                            RTm[:].rearrange("k (m p) -> k m p", p=128))
                        nc.sync.dma_start(etaC_sb[:], etaC[:])
                    if m == 16:
                        nc.sync.dma_start(bows_sb[:], bowsS[:])
                    if m >= 2:
                        emit_p1(6)
                    if m == 6:
                        h1preS = p1s.tile([B, TH], F32)
                        nc.vector.tensor_copy(h1preS[:], psA[:, :, :400])
                        nc.sync.dma_start(ar1_in[:], h1preS[:])
                        es1.close()
                        nc.gpsimd.collective_compute(
                            "AllReduce", OP.add, replica_groups=groups,
                            ins=[ar1_in[:].opt()], outs=[ar1_out[:].opt()])
                        pm = es2.enter_context(tc.tile_pool(name="pm", bufs=1))
                        pmps = es2.enter_context(
                            tc.tile_pool(name="pmps", bufs=2, space="PSUM"))
                    if m == 7:
                        # ---- MLP (executes when AR1 lands) ----
                        h1pre = pm.tile([B, TH], F32)
                        nc.sync.dma_start(h1pre[:], ar1_out[:])
                        hsum = pm.tile([B, TH], F32)
                        nc.vector.tensor_add(hsum[:], h1pre[:], etaC_sb[:])
                        h1b = pm.tile([B, TH], BF16)
                        nc.scalar.activation(h1b[:], hsum[:], AF.Relu)
                        h1T = pm.tile([128, 7, B], BF16)
                        for j in range(7):
                            ptp = pmps.tile([128, 512], BF16, name="ptp",
                                            tag="mps")
                            nc.tensor.transpose(
                                ptp[:KJ[j], :128],
                                h1b[:, j * 128:j * 128 + KJ[j]], ident[:])
                            nc.vector.tensor_copy(h1T[:KJ[j], j, :],
                                                  ptp[:KJ[j], :128])
                        w2T_sb = pm.tile([128, 7, TH], BF16)
                        for j in range(7):
                            nc.sync.dma_start(w2T_sb[:KJ[j], j, :],
                                              w2T[:KJ[j], j, :])
                        b2R_sb = pm.tile([128, 7], F32)
                        nc.sync.dma_start(b2R_sb[:], b2R[:])
                        h2T = pm.tile([128, 7, B], BF16)
                        for jo in range(7):
                            wjo = KJ[jo]
                            psH = pmps.tile([128, 512], F32, name="psH",
                                            tag="mps")
                            for ji in range(7):
                                nc.tensor.matmul(
                                    psH[:wjo, :B],
                                    w2T_sb[:KJ[ji], ji,
                                           jo * 128:jo * 128 + wjo],
                                    h1T[:KJ[ji], ji, :],
                                    start=(ji == 0), stop=(ji == 6))
                            nc.scalar.activation(h2T[:wjo, jo, :],
                                                 psH[:wjo, :B], AF.Relu,
                                                 bias=b2R_sb[:wjo, jo:jo + 1])
                        wmuls_sb = pm.tile([128, 7, 2 * K], BF16)
                        nc.sync.dma_start(wmuls_sb[:], wmulsT[:])
                        psM = pmps.tile([128, 512], F32, name="psM", tag="mps")
                        for ji in range(7):
                            nc.tensor.matmul(psM[:B, :2 * K],
                                             h2T[:KJ[ji], ji, :],
                                             wmuls_sb[:KJ[ji], ji, :],
                                             start=(ji == 0), stop=(ji == 6))
                        bmuls_sb = pm.tile([B, 2 * K], F32)
                        nc.sync.dma_start(bmuls_sb[:], bmulsB[:])
                        muls = pm.tile([B, 2 * K], F32)
                        nc.vector.tensor_add(muls[:], psM[:B, :2 * K],
                                             bmuls_sb[:])
                        mu = muls[:, :K]
                        ls = muls[:, K:]
                        sd = pm.tile([B, K], F32)
                        nc.scalar.activation(sd[:], ls, AF.Exp, scale=0.5)
                        epsTH_sb = pm.tile([B, K], F32)
                        nc.sync.dma_start(epsTH_sb[:], epsTH[:])
                        ez0 = pm.tile([B, K], F32)
                        nc.vector.tensor_mul(ez0[:], epsTH_sb[:], sd[:])
                        zt = pm.tile([B, K], F32)
                        nc.vector.tensor_add(zt[:], mu, ez0[:])
                        zm = pm.tile([B, 1], F32)
                        nc.vector.reduce_max(zm[:], zt[:],
                                             axis=mybir.AxisListType.X,
                                             negate=True)
                        et = pm.tile([B, K], F32)
                        se = pm.tile([B, 1], F32)
                        nc.scalar.activation(et[:], zt[:], AF.Exp, bias=zm[:],
                                             accum_out=se[:])
                        rse = pm.tile([B, 1], F32)
                        nc.vector.reciprocal(rse[:], se[:])
                        theta = pm.tile([B, K], F32)
                        nc.vector.tensor_scalar_mul(theta[:], et[:], rse[:])
                        # kl_theta
                        etaTD_sb = pm.tile([B, K], F32)
                        nc.sync.dma_start(etaTD_sb[:], etaTD[:])
                        sd2 = pm.tile([B, K], F32)
                        nc.vector.tensor_mul(sd2[:], sd[:], sd[:])
                        dd = pm.tile([B, K], F32)
                        nc.vector.tensor_sub(dd[:], mu, etaTD_sb[:])
                        dd2 = pm.tile([B, K], F32)
                        nc.vector.tensor_mul(dd2[:], dd[:], dd[:])
                        uu = pm.tile([B, K], F32)
                        sA = pm.tile([B, 1], F32)
                        nc.vector.scalar_tensor_tensor(
                            uu[:], dd2[:], 1.0, sd2[:],
                            op0=OP.bypass, op1=OP.add, accum_out=sA[:])
                        sB_ = pm.tile([B, 1], F32)
                        nc.vector.reduce_sum(sB_[:], ls,
                                             axis=mybir.AxisListType.X)
                        q1 = pm.tile([B, 1], F32)
                        nc.vector.tensor_scalar(q1[:], sA[:], float(RKL),
                                                -float(K) * 0.5,
                                                op0=OP.mult, op1=OP.add)
                        q2 = pm.tile([B, 1], F32)
                        nc.vector.tensor_scalar_mul(q2[:], sB_[:], 0.5)
                        klth = pm.tile([B, 1], F32)
                        nc.vector.tensor_sub(klth[:], q1[:], q2[:])
                        nc.sync.dma_start(klthOut[:], klth[:])
                    if m == 12:
                        # fire Z AllReduce for chunks 0..11 (z rows 0..1535)
                        zredA = pm.tile([128, 12], F32)
                        nc.vector.reduce_sum(
                            zredA[:],
                            zparts[:, 0:36].rearrange("p (m c) -> p m c", c=3),
                            axis=mybir.AxisListType.X)
                        zredSA = pm.tile([128, 12], F32)
                        nc.vector.tensor_scalar_mul(zredSA[:], zredA[:],
                                                    scal[:, 2:3])
                        nc.sync.dma_start(
                            z_inA[:].rearrange("(m p) -> p m", p=128),
                            zredSA[:])
                        nc.gpsimd.collective_compute(
                            "AllReduce", OP.add, replica_groups=groups,
                            ins=[z_inA[:].opt()], outs=[z_outA[:].opt()])

                # ---- Z second half (chunks 12..23, z rows 1536..3071) ----
                zredB = pm.tile([128, 12], F32)
                nc.vector.reduce_sum(
                    zredB[:],
                    zparts[:, 36:72].rearrange("p (m c) -> p m c", c=3),
                    axis=mybir.AxisListType.X)
                zredSB = pm.tile([128, 12], F32)
                nc.vector.tensor_scalar_mul(zredSB[:], zredB[:], scal[:, 2:3])
                nc.sync.dma_start(z_inB[:].rearrange("(m p) -> p m", p=128),
                                  zredSB[:])
                nc.gpsimd.collective_compute(
                    "AllReduce", OP.add, replica_groups=groups,
                    ins=[z_inB[:].opt()], outs=[z_outB[:].opt()])

                # ---- thz-A (times 0..29 exact; others -> Z=1) + G2 0..10 --
                zmatA = pm.tile([31, K], F32)
                nc.vector.memset(zmatA[:], 1.0)
                nc.sync.dma_start(zmatA[0:30, :],
                                  z_outA[0:1500].rearrange("(t k) -> t k", k=K))
                psZA = pmps.tile([128, 512], F32, name="psZ", tag="mps")
                nc.tensor.matmul(psZA[:B, :K], onehotA_sb[:], zmatA[:],
                                 start=True, stop=True)
                rzA = pm.tile([B, K], F32)
                nc.vector.reciprocal(rzA[:], psZA[:B, :K])
                thzSA = pm.tile([128, 128], BF16)
                nc.vector.memset(thzSA[:], 0.0)
                thzA = pm.tile([B, K], F32)
                nc.vector.tensor_mul(thzA[:], theta[:], rzA[:])
                nc.vector.tensor_scalar_mul(thzSA[:B, :K], thzA[:],
                                            scal[:, 3:4])
                psTA = pmps.tile([128, 512], BF16, name="psT", tag="mps")
                nc.tensor.transpose(psTA[:, :128], thzSA[:], ident[:])
                thzSTA = pm.tile([K, B], BF16)
                nc.vector.tensor_copy(thzSTA[:], psTA[:K, :128])
                for m2 in range(11):
                    psF = pmps.tile([128, 512], F32, name="psF", tag="mps")
                    nc.tensor.matmul(psF[:, :B], RT_sb[:, m2, :], thzSTA[:],
                                     start=True, stop=True)
                    nc.vector.tensor_mul(G2[:, m2, :], psF[:, :B],
                                         F2_sb[:, m2, :])

                # ---- full zmat + thz + G2 11..23 ----
                zmatF = pm.tile([T, K], F32)
                nc.sync.dma_start(zmatF[0:30, :],
                                  z_outA[0:1500].rearrange("(t k) -> t k", k=K))
                nc.sync.dma_start(
                    zmatF[30:31, 0:36],
                    z_outA[1500:1536].rearrange("(t k) -> t k", k=36))
                nc.sync.dma_start(
                    zmatF[30:31, 36:50],
                    z_outB[0:14].rearrange("(t k) -> t k", k=14))
                nc.sync.dma_start(
                    zmatF[31:60, :],
                    z_outB[14:1464].rearrange("(t k) -> t k", k=K))
                nc.sync.dma_start(zmatOut[:], zmatF[:])
                psZF = pmps.tile([128, 512], F32, name="psZ", tag="mps")
                nc.tensor.matmul(psZF[:B, :K], onehot_sb[:], zmatF[:],
                                 start=True, stop=True)
                rz = pm.tile([B, K], F32)
                nc.vector.reciprocal(rz[:], psZF[:B, :K])
                thz = pm.tile([B, K], F32)
                nc.vector.tensor_mul(thz[:], theta[:], rz[:])
                nc.sync.dma_start(thzOut[:], thz[:])
                thzS = pm.tile([128, 128], BF16)
                nc.vector.memset(thzS[:], 0.0)
                nc.vector.tensor_scalar_mul(thzS[:B, :K], thz[:], scal[:, 3:4])
                psT2 = pmps.tile([128, 512], BF16, name="psT", tag="mps")
                nc.tensor.transpose(psT2[:, :128], thzS[:], ident[:])
                thzST = pm.tile([K, B], BF16)
                nc.vector.tensor_copy(thzST[:], psT2[:K, :128])
                for m2 in range(11, 24):
                    psF = pmps.tile([128, 512], F32, name="psF", tag="mps")
                    nc.tensor.matmul(psF[:, :B], RT_sb[:, m2, :], thzST[:],
                                     start=True, stop=True)
                    nc.vector.tensor_mul(G2[:, m2, :], psF[:, :B],
                                         F2_sb[:, m2, :])
                es2.close()

            # ---------------- Phase 3: mix matmuls + nll -----------------
            with tc.tile_pool(name="p3", bufs=1) as p3, \
                 tc.tile_pool(name="p3lm", bufs=2) as p3lm, \
                 tc.tile_pool(name="p3ps", bufs=1, space="PSUM") as p3ps:
                psd = [p3ps.tile([128, 512], F32, name=f"psd{i}")
                       for i in range(8)]
                DR = mybir.MatmulPerfMode.DoubleRow

                def mix_pair(mm, start, stop):
                    for n in range(8):
                        w = NW[n]
                        nc.tensor.matmul(
                            psd[n][:, :w], G2[:, mm:mm + 2, :],
                            exs[:, mm:mm + 2, n * 512:n * 512 + w],
                            perf_mode=DR, start=start, stop=stop)

                def mix_single(mm, start, stop):
                    mrows = MCH[mm]
                    for n in range(8):
                        w = NW[n]
                        nc.tensor.matmul(
                            psd[n][:, :w], G2[:mrows, mm, :],
                            exs[:mrows, mm, n * 512:n * 512 + w],
                            start=start, stop=stop)

                for p in range(5):                 # chunks 0..9
                    mix_pair(2 * p, start=(p == 0), stop=False)
                mix_single(10, False, False)
                mix_single(11, False, False)
                for p in range(6, 12):             # chunks 12..23
                    mix_pair(2 * p, start=False, stop=(p == 11))

                for n in range(8):
                    w = NW[n]
                    lm = p3lm.tile([B, 512], F32, name="lm")
                    nc.scalar.activation(lm[:, :w], psd[n][:, :w], AF.Ln,
                                         bias=scal[:, 1:2])
                    junk = p3lm.tile([B, 512], F32, name="junk")
                    nc.vector.scalar_tensor_tensor(
                        junk[:, :w], lm[:, :w], 1.0,
                        bows_sb[:, n * 512:n * 512 + w],
                        op0=OP.bypass, op1=OP.mult,
                        accum_out=nllp[:, n:n + 1])
                nsum = p3.tile([B, 1], F32)
                nc.vector.reduce_sum(nsum[:], nllp[:],
                                     axis=mybir.AxisListType.X, negate=True)
                nc.sync.dma_start(nllOut[:], nsum[:])

    nc.compile()
    return nc


# ---------------------------------------------------------------------------
# host-side small sequential chains (fp32 numpy)
# ---------------------------------------------------------------------------

def _sigmoid(x):
    with np.errstate(over="ignore"):
        return (1.0 / (1.0 + np.exp(-x))).astype(np.float32)


def _kl_np(qm, qls, pm, pls):
    return 0.5 * np.sum(
        (np.exp(qls) + (qm - pm) ** 2) / (np.exp(pls) + 1e-6)
        - 1.0 + pls - qls, axis=-1, dtype=np.float32)


def _host_chains(inp):
    f = np.float32
    mu_a = np.asarray(inp["mu_q_alpha"], f).transpose(1, 0, 2)
    ls_a = np.asarray(inp["logsigma_q_alpha"], f).transpose(1, 0, 2)
    eps_a = np.asarray(inp["eps_alpha"], f)
    logdelta = f(np.log(f(DELTA)))
    alphas = (mu_a + eps_a * np.exp(0.5 * ls_a)).astype(f)
    kl_alpha = f(_kl_np(mu_a[0], ls_a[0], f(0.0), f(0.0)).sum()
                 + _kl_np(mu_a[1:], ls_a[1:], alphas[:-1], logdelta).sum())

    rnn_inp = np.asarray(inp["rnn_inp"], f)
    Wmap = np.asarray(inp["Wmap"], f)
    bmap = np.asarray(inp["bmap"], f)
    out = (rnn_inp @ Wmap.T + bmap).astype(f)
    Wih = np.asarray(inp["lstm_Wih"], f)
    Whh = np.asarray(inp["lstm_Whh"], f)
    bih = np.asarray(inp["lstm_bih"], f)
    bhh = np.asarray(inp["lstm_bhh"], f)
    for l in range(L):
        h = np.zeros(H, f)
        c = np.zeros(H, f)
        pre = (out @ Wih[l].T + (bih[l] + bhh[l])).astype(f)
        ys = np.empty((T, H), f)
        for t in range(T):
            g = pre[t] + Whh[l] @ h
            i_, f_, g_, o_ = np.split(g, 4)
            c = _sigmoid(f_) * c + _sigmoid(i_) * np.tanh(g_)
            h = (_sigmoid(o_) * np.tanh(c)).astype(f)
            ys[t] = h
        out = ys
    Wmu_e = np.asarray(inp["Wmu_e"], f)
    bmu_e = np.asarray(inp["bmu_e"], f)
    Wls_e = np.asarray(inp["Wls_e"], f)
    bls_e = np.asarray(inp["bls_e"], f)
    eps_eta = np.asarray(inp["eps_eta"], f)
    inp0 = np.concatenate([out[0], np.zeros(K, f)])
    mu0 = Wmu_e @ inp0 + bmu_e
    ls0 = Wls_e @ inp0 + bls_e
    eta = mu0 + eps_eta[0] * np.exp(0.5 * ls0)
    kl_eta = _kl_np(mu0, ls0, f(0.0), f(0.0))
    etas = np.empty((T, K), f)
    etas[0] = eta
    for t in range(1, T):
        it = np.concatenate([out[t], eta])
        mu_t = Wmu_e @ it + bmu_e
        ls_t = Wls_e @ it + bls_e
        eta = (mu_t + eps_eta[t] * np.exp(0.5 * ls_t)).astype(f)
        kl_eta = kl_eta + _kl_np(mu_t, ls_t, etas[t - 1], logdelta)
        etas[t] = eta
    return alphas, f(kl_alpha), etas, f(kl_eta)


def kernel(**inputs):
    f = np.float32
    bf = ml_dtypes.bfloat16
    if "nc" not in _CACHE:
        _CACHE["nc"] = _build_program()
    nc = _CACHE["nc"]

    bows = np.asarray(inputs["bows"], f)
    nb = np.asarray(inputs["normalized_bows"], f)
    times = np.asarray(inputs["times"]).astype(np.int64)
    num_docs = float(np.asarray(inputs["num_docs"]))
    W1 = np.asarray(inputs["W1"], f)
    b1 = np.asarray(inputs["b1"], f)
    W2 = np.asarray(inputs["W2"], f)
    b2 = np.asarray(inputs["b2"], f)
    Wmu_t = np.asarray(inputs["Wmu_t"], f)
    bmu_t = np.asarray(inputs["bmu_t"], f)
    Wls_t = np.asarray(inputs["Wls_t"], f)
    bls_t = np.asarray(inputs["bls_t"], f)
    rho = np.asarray(inputs["rho"], f)
    eps_theta = np.asarray(inputs["eps_theta"], f)

    alphas, kl_alpha, etas, kl_eta = _host_chains(inputs)
    eta_td = etas[times]                                   # [B, K]
    etaC = (eta_td @ W1[:, V:].T + b1).astype(f)           # [B, TH]

    # fp8 scaling: sampled logit max -> global scale s; q for the G2 side
    alf = np.ascontiguousarray(alphas.reshape(TK, E))
    samp = np.linspace(0, V - 1, 512).astype(np.int64)
    logit_s = alf @ rho[samp].T                            # [TK, 512]
    gmax = float(logit_s.max())
    lns = gmax - 6.0
    s = np.exp(np.float64(lns))
    zest = np.exp(logit_s.astype(np.float64) - lns).mean(axis=1) * V  # ~Z/s
    zmin_est = max(float(zest.min()) * s, 1e-30)
    q = int(np.clip(np.floor(np.log2(32.0 * zmin_est / s)), -30, 40))
    scal4 = np.zeros((128, 4), f)
    scal4[:, 0] = f(-lns)
    scal4[:, 1] = f(1e-6 * (2.0 ** q))
    scal4[:, 2] = f(s)
    scal4[:, 3] = f(s * (2.0 ** q))

    onehotT = (times[None, :] == np.arange(T)[:, None]).astype(f)
    onehotA31v = np.concatenate(
        [onehotT[:30], (times[None, :] >= 30).astype(f)], axis=0)
    rows = np.arange(24 * 128)
    tgrid = rows // 50                                     # time of row (>=60 pad)
    kgrid = rows % 50
    F2m = (times[None, :] == tgrid[:, None]).astype(bf)    # [3072, B]
    F2m = np.ascontiguousarray(
        F2m.reshape(24, 128, B).transpose(1, 0, 2).reshape(128, 24 * B))
    RTm = (np.arange(50)[:, None] == kgrid[None, :]).astype(bf)  # [50, 3072]
    RTm = np.ascontiguousarray(
        RTm.reshape(50, 24, 128).reshape(50, 24 * 128))

    f8 = ml_dtypes.float8_e5m2
    alphasT = np.ascontiguousarray(alf.T).astype(f8)       # [E, TK]
    w2 = np.ascontiguousarray(W2.T).astype(bf)             # [TH(i), TH(o)]
    w2T = np.zeros((128, 7, TH), bf)
    for j in range(7):
        w2T[:KJ[j], j, :] = w2[j * 128:j * 128 + KJ[j], :]
    wmuls = np.concatenate([Wmu_t, Wls_t], axis=0).T.astype(bf)  # [TH, 2K]
    wmulsT = np.zeros((128, 7, 2 * K), bf)
    for j in range(7):
        wmulsT[:KJ[j], j, :] = wmuls[j * 128:j * 128 + KJ[j], :]
    b2R = np.zeros((128, 7), f)
    for j in range(7):
        b2R[:KJ[j], j] = b2[j * 128:j * 128 + KJ[j]]
    bmulsB = np.ascontiguousarray(
        np.broadcast_to(np.concatenate([bmu_t, bls_t]).astype(f), (B, 2 * K)))

    in_maps = []
    for c in range(NCORES):
        sl = slice(c * VS, (c + 1) * VS)
        in_maps.append({
            "nbT": np.ascontiguousarray(nb[:, sl].T).astype(bf),
            "w1vT": np.ascontiguousarray(W1[:, sl].T).astype(bf),
            "rhoT": np.ascontiguousarray(rho[sl, :].T).astype(f8),
            "alphasT": alphasT,
            "bowsS": np.ascontiguousarray(bows[:, sl]).astype(bf),
            "etaC": etaC,
            "etaTD": np.ascontiguousarray(eta_td.astype(f)),
            "epsTH": eps_theta,
            "onehotT": onehotT,
            "onehotA31": onehotA31v,
            "F2m": F2m,
            "RTm": RTm,
            "w2T": w2T,
            "wmulsT": wmulsT,
            "b2R": b2R,
            "bmulsB": bmulsB,
            "scal4": scal4,
        })

    global _LAST_IN_MAPS
    _LAST_IN_MAPS = in_maps
    res = bass_utils.run_bass_kernel_spmd(nc, in_maps,
                                          core_ids=list(range(NCORES)))
    _CACHE["res"] = res
    coeff = f(num_docs / B)
    nll_raw = sum(r["nllOut"].sum(dtype=np.float64) for r in res.results)
    # device loglik' = loglik_true + q ln2  =>  nll_true = nll_dev + q ln2 * sum(bows)
    nll_tot = f((nll_raw + q * np.log(2.0) * bows.sum(dtype=np.float64)) * coeff)
    klth_tot = f(res.results[0]["klthOut"].sum(dtype=np.float64) * coeff)
    nelbo = f(nll_tot + kl_alpha + kl_eta + klth_tot)
    return np.array([nelbo, nll_tot, kl_alpha, kl_eta, klth_tot], dtype=f)
